# revision 1
# baseline (speedup 1.0000x reference)
"""Trainium2 Bass kernel for channel ("transposed") attention:
  qkv = conv3x3(conv1x1(x)); per-head L2-normalized channel attention; 1x1 proj.

Sharding: pure data-parallel — batch 8 across 8 NeuronCores (one image each).
Per-core pipeline (all matmuls bf16 with f32 PSUM accumulation):
  A: y1p = w1 @ xp (host-padded input, 130x130) -> DRAM (bf16)
  B: conv3x3 as 9 shifted matmuls x 5 k-tiles accumulated in PSUM;
     v kept SBUF-resident, q/k streamed to DRAM + squared-norm reduce;
     per-n-tile qk DMA-transposes and per-head logit matmuls interleaved
     (one tile lagged) so the PE stream never breaks
  C: norm/scale softmax on the tiny [48,48] logits; FW = wproj @ blockdiag(A)
  D: out = FW @ v -> f32 output

All K=64 remainder matmuls are zero-padded to K=128 (host-padded weights,
zero-filled y1p tail rows) so every LDWEIGHTS is a uniform 128-row load.
"""
import numpy as np
import ml_dtypes

import concourse.bass as bass
import concourse.tile as tile
from concourse import bacc, mybir
from concourse.bass_utils import run_bass_kernel_spmd

BF16NP = ml_dtypes.bfloat16
BF = mybir.dt.bfloat16
F32 = mybir.dt.float32

B, C, H, W = 8, 192, 128, 128
HEADS, CH = 4, 48
C3 = 3 * C                      # 576
C3P = 640                       # 576 padded to 5*128
CP2 = 256                       # 192 padded to 2*128
HP, WP = H + 2, W + 2           # 130
NPIX = H * W                    # 16384
NPP = HP * WP                   # 16900
NT = 512
NTB = NPIX // NT                # 32 conv n-tiles
NTA = 34                        # stage-A n-tiles (33x512 + 1 overlapping)
KT = [(0, 128), (128, 128), (256, 128), (384, 128), (512, 64)]   # 576 split
MT_OUT = [(0, 128), (128, 64)]                                   # out-ch split

_CACHE = {}


def _build(variant="full"):
    nc = bacc.Bacc("TRN2", target_bir_lowering=False, debug=False, num_devices=8)
    xp_d = nc.dram_tensor("xp", [CP2, NPP], BF, kind="ExternalInput").ap()
    w1t_d = nc.dram_tensor("w1t", [CP2, C3], BF, kind="ExternalInput").ap()
    w2sb_d = nc.dram_tensor("w2sb", [5, 128, 9 * C3], BF, kind="ExternalInput").ap()
    wpt_d = nc.dram_tensor("wpt", [CH, HEADS * C], BF, kind="ExternalInput").ap()
    scale_d = nc.dram_tensor("scale", [1, HEADS], F32, kind="ExternalInput").ap()
    out_d = nc.dram_tensor("out", [C, NPIX], F32, kind="ExternalOutput").ap()

    with tile.TileContext(nc) as tc:
        with tc.tile_pool(name="dram", bufs=1, space="DRAM") as dram:
            y1p = dram.tile([C3P, NPP], BF)
            qk = dram.tile([2 * C, NPIX], BF)
            rinv_d = dram.tile([1, 512], F32)
            fwt_d = dram.tile([C, C], BF)
            _build_body(nc, tc, xp_d, w1t_d, w2sb_d, wpt_d, scale_d, out_d,
                        y1p, qk, rinv_d, fwt_d, variant)
    nc.compile()
    return nc


def _build_body(nc, tc, xp_d, w1t_d, w2sb_d, wpt_d, scale_d, out_d,
                y1p, qk, rinv_d, fwt_d, variant="full"):
    X = mybir.AxisListType.X

    with tc.tile_pool(name="persist", bufs=1) as persist:
        v0 = persist.tile([128, NPIX], BF, tag="v0")
        v1 = persist.tile([64, NPIX], BF, tag="v1")
        parts = [persist.tile([mp, NTB], F32, tag=f"part{i}", name=f"part{i}")
                 for i, (m0, mp) in enumerate(KT[:3])]
        psG_ctx = tc.tile_pool(name="psG", bufs=1, space="PSUM")
        psG = psG_ctx.__enter__()
        Gall = None

        # ---------------- Phase A + B (+ interleaved logits) ----------------
        with (tc.tile_pool(name="wts", bufs=1) as wts,
              tc.tile_pool(name="xk", bufs=6) as xkp,
              tc.tile_pool(name="slab", bufs=10) as slabp,
              tc.tile_pool(name="stage", bufs=8) as stagep,
              tc.tile_pool(name="sq", bufs=3) as sqp,
              tc.tile_pool(name="qkt", bufs=8) as qktp,
              tc.tile_pool(name="psA", bufs=6, space="PSUM") as psA):

            w1s = []
            for i in range(2):
                t = wts.tile([128, C3], BF, tag=f"w1_{i}", name=f"w1_{i}")
                nc.sync.dma_start(t[:], w1t_d[128 * i:128 * (i + 1), :])
                w1s.append(t)
            w2s = []
            for i in range(5):
                t = wts.tile([128, 9 * C3], BF, tag=f"w2_{i}", name=f"w2_{i}")
                nc.sync.dma_start(t[:], w2sb_d[i, :, :])
                w2s.append(t)

            # zero-fill y1p rows 576..640 once (K=128 padding for conv kt=4)
            zst = wts.tile([64, 2048], BF, tag="zst")
            nc.vector.memset(zst[:], 0.0)
            for a in range(NPP // 2048 + 1):
                o = a * 2048 if a < NPP // 2048 else NPP - 2048
                nc.sync.dma_start(y1p[C3:C3P, o:o + 2048], zst[:])

            # Phase A: y1p = w1 @ xp
            for t in range(NTA):
                off = t * NT if t < NTA - 1 else NPP - NT
                xks = []
                for i in range(2):
                    xk = xkp.tile([128, NT], BF, tag="xk")
                    nc.sync.dma_start(xk[:], xp_d[128 * i:128 * (i + 1), off:off + NT])
                    xks.append(xk)
                for mi, (m0, mp) in enumerate(KT):
                    ps = psA.tile([128, NT], F32, tag="ps")
                    for i in range(2):
                        nc.tensor.matmul(ps[:mp], w1s[i][:, m0:m0 + mp],
                                         xks[i][:], start=(i == 0), stop=(i == 1))
                    st = stagep.tile([128, NT], BF, tag="stage")
                    if mi % 2 == 0:
                        nc.vector.tensor_copy(st[:mp], ps[:mp])
                    else:
                        nc.scalar.copy(st[:mp], ps[:mp])
                    nc.sync.dma_start(y1p[m0:m0 + mp, off:off + NT], st[:mp])

            # logits PSUM accumulator: 4 heads packed in one bank [48, 192]
            Gall = psG.tile([CH, HEADS * CH], F32, tag="Gall")

            def issue_logits(tt):
                """4 chunk-transposes for conv n-tile tt were issued after
                tile tt's groups; the G matmuls for tile tt are issued here
                (one tile later) so the transpose DMA has a full tile of
                compute to hide under."""
                for j in range(4 * tt, 4 * tt + 4):
                    qkt = qktp.tile([128, 2 * C], BF, tag="qkt", name="qkt")
                    nc.sync.dma_start_transpose(qkt[:], qk[:, j * 128:(j + 1) * 128])
                    for h in range(HEADS):
                        nc.tensor.matmul(
                            Gall[:, CH * h:CH * (h + 1)],
                            qkt[:, CH * h:CH * (h + 1)],
                            qkt[:, C + CH * h:C + CH * (h + 1)],
                            start=(j == 0 and h == 0),
                            stop=(j == 127 and h == HEADS - 1),
                            skip_group_check=True)

            # Phase B: conv3x3 via 9 shifted matmuls (+ lagged logits)
            y1p_img = y1p.rearrange("c (h w) -> c h w", h=HP)
            for t in range(NTB):
                slabs = []
                for i, (k0, kp) in enumerate(KT):
                    sl = slabp.tile([128, 6, WP], BF, tag="slab")
                    nc.sync.dma_start(sl[:], y1p_img[128 * i:128 * (i + 1),
                                                     4 * t:4 * t + 6, :])
                    slabs.append(sl)
                for mi, (m0, mp) in enumerate(KT):
                    ps = psA.tile([128, NT], F32, tag="ps")
                    n_mm = 0
                    for s in range(9):
                        dy, dx = s // 3, s % 3
                        for i in range(5):
                            nc.tensor.matmul(
                                ps[:mp],
                                w2s[i][:, s * C3 + m0: s * C3 + m0 + mp],
                                slabs[i][:, dy:dy + 4, dx:dx + W],
                                start=(n_mm == 0), stop=(n_mm == 44))
                            n_mm += 1
                    if mi >= 3:   # v channels -> SBUF resident
                        vt, vp = (v0, 128) if mi == 3 else (v1, 64)
                        if mi == 3:
                            nc.scalar.copy(vt[:vp, t * NT:(t + 1) * NT], ps[:vp])
                        else:
                            nc.vector.tensor_copy(vt[:vp, t * NT:(t + 1) * NT], ps[:vp])
                        if variant == "ab":
                            stf = stagep.tile([128, NT], F32, tag="stagef",
                                              name="stf")
                            nc.any.tensor_copy(stf[:mp], ps[:mp])
                            nc.sync.dma_start(
                                out_d[m0 - 384:m0 - 384 + mp, t * NT:(t + 1) * NT],
                                stf[:mp])
                    else:         # q/k channels -> DRAM + norm partials
                        st = stagep.tile([128, NT], BF, tag="stage")
                        if mi % 2 == 0:
                            nc.vector.tensor_copy(st[:mp], ps[:mp])
                        else:
                            nc.scalar.copy(st[:mp], ps[:mp])
                        sq = sqp.tile([128, NT], F32, tag="sq")
                        nc.vector.tensor_mul(sq[:mp], st[:mp], st[:mp])
                        nc.vector.reduce_sum(parts[mi][:mp, t:t + 1], sq[:mp], axis=X)
                        nc.sync.dma_start(qk[m0:m0 + mp, t * NT:(t + 1) * NT], st[:mp])
                if variant != "ab":
                    # transposes for tile t (DMA, hides under tile t+1 compute)
                    for j in range(4 * t, 4 * t + 4):
                        pass  # issued inside issue_logits with 1-tile lag
                    if t >= 1:
                        issue_logits(t - 1)
            if variant == "ab":
                return
            issue_logits(NTB - 1)

        # -------- Phase C (small): norms, softmax, FW --------
        with (tc.tile_pool(name="small", bufs=1) as smallp,
              tc.tile_pool(name="soft", bufs=2) as softp,
              tc.tile_pool(name="psF", bufs=1, space="PSUM") as psF):
            for mi, (m0, mp) in enumerate(KT[:3]):
                ssq = smallp.tile([128, 1], F32, tag=f"ssq{mi}", name=f"ssq{mi}")
                nc.vector.reduce_sum(ssq[:mp], parts[mi][:mp, :], axis=X)
                nc.scalar.sqrt(ssq[:mp], ssq[:mp])
                nc.vector.reciprocal(ssq[:mp], ssq[:mp])
                nc.sync.dma_start(rinv_d[0, 128 * mi:128 * mi + mp], ssq[:mp, 0])

            alpha = smallp.tile([CH, HEADS], F32, tag="alpha")
            for h in range(HEADS):
                nc.sync.dma_start(alpha[:, h:h + 1], rinv_d[0, CH * h:CH * (h + 1)])
            scs = smallp.tile([CH, HEADS], F32, tag="scs")
            nc.gpsimd.dma_start(out=scs[:],
                                in_=scale_d[0:1, :].to_broadcast((CH, HEADS)))
            nc.vector.tensor_mul(alpha[:], alpha[:], scs[:])
            ball = smallp.tile([CH, C], F32, tag="ball")
            nc.gpsimd.dma_start(out=ball[:],
                                in_=rinv_d[0:1, C:2 * C].to_broadcast((CH, C)))

            wpt_sb = smallp.tile([CH, HEADS * C], BF, tag="wpt")
            nc.sync.dma_start(wpt_sb[:], wpt_d[:, :])

            for h in range(HEADS):
                gh = Gall[:, CH * h:CH * (h + 1)]
                nc.vector.tensor_scalar_mul(gh, gh, alpha[:, h:h + 1])
                gsb = softp.tile([CH, CH], F32, tag="gsb")
                nc.vector.tensor_mul(gsb[:], gh, ball[:, CH * h:CH * (h + 1)])
                mx = softp.tile([CH, 1], F32, tag="mx")
                nc.vector.reduce_max(mx[:], gsb[:], axis=X)
                nc.vector.tensor_scalar_mul(mx[:], mx[:], -1.0)
                ex = softp.tile([CH, CH], F32, tag="ex")
                nc.scalar.activation(ex[:], gsb[:],
                                     mybir.ActivationFunctionType.Exp,
                                     bias=mx[:], scale=1.0)
                sm = softp.tile([CH, 1], F32, tag="sm")
                nc.vector.reduce_sum(sm[:], ex[:], axis=X)
                nc.vector.reciprocal(sm[:], sm[:])
                asb = softp.tile([CH, CH], BF, tag="asb")
                nc.vector.tensor_scalar_mul(asb[:], ex[:], sm[:, 0:1])
                fw_ps = psF.tile([CH, C], F32, tag="fw")
                nc.tensor.matmul(fw_ps[:], asb[:], wpt_sb[:, C * h:C * (h + 1)],
                                 start=True, stop=True)
                fw_sb = softp.tile([CH, C], BF, tag="fwsb")
                nc.any.tensor_copy(fw_sb[:], fw_ps[:])
                nc.sync.dma_start(fwt_d[CH * h:CH * (h + 1), :], fw_sb[:])

            fwt0 = smallp.tile([128, C], BF, tag="fwt0")
            nc.sync.dma_start(fwt0[:], fwt_d[0:128, :])
            fwt1 = smallp.tile([64, C], BF, tag="fwt1")
            nc.sync.dma_start(fwt1[:], fwt_d[128:C, :])

            # -------- Phase D: out = FW @ v --------
            with (tc.tile_pool(name="ostage", bufs=6) as ostagep,
                  tc.tile_pool(name="psD", bufs=4, space="PSUM") as psD):
                for t in range(NTB):
                    for oi, (m0, mp) in enumerate(MT_OUT):
                        ps = psD.tile([128, NT], F32, tag="psD")
                        nc.tensor.matmul(ps[:mp], fwt0[:, m0:m0 + mp],
                                         v0[:, t * NT:(t + 1) * NT],
                                         start=True, stop=False)
                        nc.tensor.matmul(ps[:mp], fwt1[:, m0:m0 + mp],
                                         v1[:, t * NT:(t + 1) * NT],
                                         start=False, stop=True)
                        ost = ostagep.tile([128, NT], F32, tag="ost")
                        if (t + oi) % 2 == 0:
                            nc.vector.tensor_copy(ost[:mp], ps[:mp])
                        else:
                            nc.scalar.copy(ost[:mp], ps[:mp])
                        nc.sync.dma_start(out_d[m0:m0 + mp, t * NT:(t + 1) * NT],
                                          ost[:mp])
        psG_ctx.__exit__(None, None, None)


def _prep_shared(w_qkv1, w_qkv2, w_proj, scale):
    w1t = np.zeros((CP2, C3), dtype=BF16NP)
    w1t[:C] = np.ascontiguousarray(w_qkv1[:, :, 0, 0].T).astype(BF16NP)
    w2t = np.transpose(w_qkv2, (2, 3, 1, 0)).reshape(9, C3, C3)          # [s,i,o]
    w2sb = np.zeros((5, 128, 9 * C3), dtype=BF16NP)
    for kt, (k0, kp) in enumerate(KT):
        w2sb[kt, :kp, :] = np.ascontiguousarray(
            np.transpose(w2t[:, k0:k0 + kp, :], (1, 0, 2)).reshape(kp, 9 * C3)
        ).astype(BF16NP)
    wpf = w_proj[:, :, 0, 0].T                                            # [c,o]
    wpt = np.concatenate([wpf[h * CH:(h + 1) * CH, :] for h in range(HEADS)],
                         axis=1).astype(BF16NP)                           # [48,768]
    sc = np.asarray(scale, np.float32).reshape(1, HEADS)
    return w1t, w2sb, wpt, sc


def kernel(x, w_qkv1, w_qkv2, w_proj, scale):
    x = np.asarray(x, np.float32)
    if "nc" not in _CACHE:
        _CACHE["nc"] = _build()
    nc = _CACHE["nc"]

    w1t, w2sb, wpt, sc = _prep_shared(
        np.asarray(w_qkv1, np.float32), np.asarray(w_qkv2, np.float32),
        np.asarray(w_proj, np.float32), np.asarray(scale, np.float32))

    xp = np.zeros((B, CP2, HP, WP), np.float32)
    xp[:, :C, 1:H + 1, 1:W + 1] = x
    xp = xp.astype(BF16NP).reshape(B, CP2, NPP)

    in_maps = [{"xp": xp[i], "w1t": w1t, "w2sb": w2sb, "wpt": wpt, "scale": sc}
               for i in range(B)]
    res = run_bass_kernel_spmd(nc, in_maps, core_ids=list(range(B)))
    out = np.stack([res.results[i]["out"].reshape(C, H, W) for i in range(B)], 0)
    return np.ascontiguousarray(out.astype(np.float32))



# revision 2
# speedup vs baseline: 1.2429x; 1.2429x over previous
"""Trainium2 Bass kernel for channel ("transposed") attention:
  qkv = conv3x3(conv1x1(x)); per-head L2-normalized channel attention; 1x1 proj.

Sharding: pure data-parallel — batch 8 across 8 NeuronCores (one image each).
Per-core pipeline (matmuls accumulate in f32 PSUM):
  A: y1p = w1 @ xp (host-padded input, 130x130) -> DRAM (bf16)
  B: conv3x3 as 9 shifted matmuls; q/k output channels (0:384) use fp8e4
     DoubleRow matmuls (2 input-subtile pairs + 1 plain fp8 matmul per
     shift; weights host-scaled x64 to avoid e4m3 subnormals — the scale
     cancels in the q/k L2 normalization), v channels (384:576) stay bf16;
     v kept SBUF-resident, q/k streamed to DRAM + squared-norm reduce;
     per-n-tile qk DMA-transposes and per-head logit matmuls interleaved
     (one tile lagged) so the PE stream never breaks
  C: norm/scale softmax on the tiny [48,48] logits; FW = wproj @ blockdiag(A)
  D: out = FW @ v -> f32 output
"""
import numpy as np
import ml_dtypes

import concourse.bass as bass
import concourse.tile as tile
from concourse import bacc, mybir
from concourse.bass_utils import run_bass_kernel_spmd

BF16NP = ml_dtypes.bfloat16
F8NP = ml_dtypes.float8_e4m3
BF = mybir.dt.bfloat16
F8 = mybir.dt.float8e4
F32 = mybir.dt.float32
DR = mybir.MatmulPerfMode.DoubleRow

B, C, H, W = 8, 192, 128, 128
HEADS, CH = 4, 48
C3 = 3 * C                      # 576
C3P = 640                       # 576 padded to 5*128
CP2 = 256                       # 192 padded to 2*128
HP, WP = H + 2, W + 2           # 130
NPIX = H * W                    # 16384
NPP = HP * WP                   # 16900
NT = 512
NTB = NPIX // NT                # 32 conv n-tiles
NTA = 34                        # stage-A n-tiles (33x512 + 1 overlapping)
KT = [(0, 128), (128, 128), (256, 128), (384, 128), (512, 64)]   # 576 split
MT_OUT = [(0, 128), (128, 64)]                                   # out-ch split
CQK = 384                       # q/k output channels
CV = 192                        # v output channels
W8SCALE = 64.0                  # fp8 q/k weight prescale (cancels in L2 norm)

_CACHE = {}


def _build(variant="full"):
    nc = bacc.Bacc("TRN2", target_bir_lowering=False, debug=False, num_devices=8)
    xp_d = nc.dram_tensor("xp", [CP2, NPP], BF, kind="ExternalInput").ap()
    w1t_d = nc.dram_tensor("w1t", [CP2, C3], BF, kind="ExternalInput").ap()
    w2v_d = nc.dram_tensor("w2v", [5, 128, 9 * CV], BF, kind="ExternalInput").ap()
    w8p_d = nc.dram_tensor("w8p", [2, 128, 2 * 9 * CQK], F8, kind="ExternalInput").ap()
    w8s_d = nc.dram_tensor("w8s", [128, 9 * CQK], F8, kind="ExternalInput").ap()
    wpt_d = nc.dram_tensor("wpt", [CH, HEADS * C], BF, kind="ExternalInput").ap()
    scale_d = nc.dram_tensor("scale", [1, HEADS], F32, kind="ExternalInput").ap()
    out_d = nc.dram_tensor("out", [C, NPIX], F32, kind="ExternalOutput").ap()

    with tile.TileContext(nc) as tc:
        with tc.tile_pool(name="dram", bufs=1, space="DRAM") as dram:
            y1p = dram.tile([C3P, NPP], BF)
            qk = dram.tile([2 * C, NPIX], BF)
            rinv_d = dram.tile([1, 512], F32)
            fwt_d = dram.tile([C, C], BF)
            _build_body(nc, tc, xp_d, w1t_d, w2v_d, w8p_d, w8s_d, wpt_d,
                        scale_d, out_d, y1p, qk, rinv_d, fwt_d, variant)
    nc.compile()
    return nc


def _build_body(nc, tc, xp_d, w1t_d, w2v_d, w8p_d, w8s_d, wpt_d, scale_d,
                out_d, y1p, qk, rinv_d, fwt_d, variant="full"):
    X = mybir.AxisListType.X

    with tc.tile_pool(name="persist", bufs=1) as persist:
        v0 = persist.tile([128, NPIX], BF, tag="v0")
        v1 = persist.tile([64, NPIX], BF, tag="v1")
        parts = [persist.tile([mp, NTB], F32, tag=f"part{i}", name=f"part{i}")
                 for i, (m0, mp) in enumerate(KT[:3])]
        psG_ctx = tc.tile_pool(name="psG", bufs=1, space="PSUM")
        psG = psG_ctx.__enter__()
        Gall = None

        # ---------------- Phase A + B (+ interleaved logits) ----------------
        with (tc.tile_pool(name="wts", bufs=1) as wts,
              tc.tile_pool(name="xk", bufs=12) as xkp,
              tc.tile_pool(name="slab", bufs=10) as slabp,
              tc.tile_pool(name="slab8", bufs=3) as slab8p,
              tc.tile_pool(name="stage", bufs=8) as stagep,
              tc.tile_pool(name="sq", bufs=3) as sqp,
              tc.tile_pool(name="qkt", bufs=8) as qktp,
              tc.tile_pool(name="psA", bufs=6, space="PSUM") as psA):

            # w1 first (phase A needs it immediately)
            w1s = []
            for i in range(2):
                t = wts.tile([128, C3], BF, tag=f"w1_{i}", name=f"w1_{i}")
                nc.sync.dma_start(t[:], w1t_d[128 * i:128 * (i + 1), :])
                w1s.append(t)
            # conv3x3 weights: tiles declared here, DMAs issued after phase A
            # loop (they're only needed in phase B; keeps the queue head free)
            w2vs = [wts.tile([128, 9 * CV], BF, tag=f"w2v_{i}", name=f"w2v_{i}")
                    for i in range(5)]
            w8ps = [wts.tile([128, 2, 9 * CQK], F8, tag=f"w8p_{i}", name=f"w8p_{i}")
                    for i in range(2)]
            w8ss = wts.tile([128, 9 * CQK], F8, tag="w8s", name="w8s")
            zst = wts.tile([64, 2048], BF, tag="zst")

            # Phase A: y1p = w1 @ xp
            for t in range(NTA):
                off = t * NT if t < NTA - 1 else NPP - NT
                xks = []
                for i in range(2):
                    xk = xkp.tile([128, NT], BF, tag="xk")
                    nc.sync.dma_start(xk[:], xp_d[128 * i:128 * (i + 1), off:off + NT])
                    xks.append(xk)
                for mi, (m0, mp) in enumerate(KT):
                    ps = psA.tile([128, NT], F32, tag="ps")
                    for i in range(2):
                        nc.tensor.matmul(ps[:mp], w1s[i][:, m0:m0 + mp],
                                         xks[i][:], start=(i == 0), stop=(i == 1))
                    st = stagep.tile([128, NT], BF, tag="stage")
                    if mi % 2 == 0:
                        nc.vector.tensor_copy(st[:mp], ps[:mp])
                    else:
                        nc.scalar.copy(st[:mp], ps[:mp])
                    nc.sync.dma_start(y1p[m0:m0 + mp, off:off + NT], st[:mp])
                if t == 0:
                    # phase-B weights: stream in under phase A's compute
                    for i in range(5):
                        nc.sync.dma_start(w2vs[i][:], w2v_d[i, :, :])
                    for i in range(2):
                        nc.sync.dma_start(w8ps[i][:], w8p_d[i, :, :])
                    nc.sync.dma_start(w8ss[:], w8s_d[:, :])
                    # zero-fill y1p rows 576..640 (K padding for conv kt=4)
                    nc.vector.memset(zst[:], 0.0)
                    for a in range(NPP // 2048 + 1):
                        o = a * 2048 if a < NPP // 2048 else NPP - 2048
                        nc.sync.dma_start(y1p[C3:C3P, o:o + 2048], zst[:])

            # logits PSUM accumulator: 4 heads packed in one bank [48, 192]
            Gall = psG.tile([CH, HEADS * CH], F32, tag="Gall")

            def issue_logits(tt):
                """4 chunk-transposes for conv n-tile tt were issued after
                tile tt's groups; the G matmuls for tile tt are issued here
                (one tile later) so the transpose DMA has a full tile of
                compute to hide under."""
                for j in range(4 * tt, 4 * tt + 4):
                    qkt = qktp.tile([128, 2 * C], BF, tag="qkt", name="qkt")
                    nc.sync.dma_start_transpose(qkt[:], qk[:, j * 128:(j + 1) * 128])
                    for h in range(HEADS):
                        nc.tensor.matmul(
                            Gall[:, CH * h:CH * (h + 1)],
                            qkt[:, CH * h:CH * (h + 1)],
                            qkt[:, C + CH * h:C + CH * (h + 1)],
                            start=(j == 0 and h == 0),
                            stop=(j == 127 and h == HEADS - 1),
                            skip_group_check=True)

            # Phase B: conv3x3 via 9 shifted matmuls (+ lagged logits)
            # q/k m-tiles (mi 0..2) in fp8 DoubleRow, v m-tiles (3,4) in bf16.
            y1p_img = y1p.rearrange("c (h w) -> c h w", h=HP)
            for t in range(NTB):
                slabs = []
                for i, (k0, kp) in enumerate(KT):
                    sl = slabp.tile([128, 6, WP], BF, tag="slab")
                    nc.sync.dma_start(sl[:], y1p_img[128 * i:128 * (i + 1),
                                                     4 * t:4 * t + 6, :])
                    slabs.append(sl)
                # fp8 copy of all 5 input subtiles: [128, 5, 6, WP]
                s8 = slab8p.tile([128, 5, 6, WP], F8, tag="s8")
                for i in range(5):
                    if i % 2 == 0:
                        nc.vector.tensor_copy(s8[:, i], slabs[i][:])
                    else:
                        nc.scalar.copy(s8[:, i], slabs[i][:])
                for mi, (m0, mp) in enumerate(KT):
                    ps = psA.tile([128, NT], F32, tag="ps")
                    if mi < 3:      # q/k channels: fp8 DoubleRow
                        n_mm = 0
                        for s in range(9):
                            dy, dx = s // 3, s % 3
                            for pp in range(2):
                                nc.tensor.matmul(
                                    ps[:mp],
                                    w8ps[pp][:, :, s * CQK + m0: s * CQK + m0 + mp],
                                    s8[:, 2 * pp:2 * pp + 2, dy:dy + 4, dx:dx + W],
                                    start=(n_mm == 0), stop=False,
                                    perf_mode=DR)
                                n_mm += 1
                            nc.tensor.matmul(
                                ps[:mp],
                                w8ss[:, s * CQK + m0: s * CQK + m0 + mp],
                                s8[:, 4, dy:dy + 4, dx:dx + W],
                                start=False, stop=(s == 8))
                            n_mm += 1
                    else:           # v channels: bf16
                        n_mm = 0
                        for s in range(9):
                            dy, dx = s // 3, s % 3
                            for i in range(5):
                                nc.tensor.matmul(
                                    ps[:mp],
                                    w2vs[i][:, s * CV + m0 - CQK:
                                            s * CV + m0 - CQK + mp],
                                    slabs[i][:, dy:dy + 4, dx:dx + W],
                                    start=(n_mm == 0), stop=(n_mm == 44))
                                n_mm += 1
                    if mi >= 3:   # v channels -> SBUF resident
                        vt, vp = (v0, 128) if mi == 3 else (v1, 64)
                        if mi == 3:
                            nc.scalar.copy(vt[:vp, t * NT:(t + 1) * NT], ps[:vp])
                        else:
                            nc.vector.tensor_copy(vt[:vp, t * NT:(t + 1) * NT], ps[:vp])
                        if variant == "ab":
                            stf = stagep.tile([128, NT], F32, tag="stagef",
                                              name="stf")
                            nc.any.tensor_copy(stf[:mp], ps[:mp])
                            nc.sync.dma_start(
                                out_d[m0 - 384:m0 - 384 + mp, t * NT:(t + 1) * NT],
                                stf[:mp])
                    else:         # q/k channels -> DRAM + norm partials
                        st = stagep.tile([128, NT], BF, tag="stage")
                        if mi % 2 == 0:
                            nc.vector.tensor_copy(st[:mp], ps[:mp])
                        else:
                            nc.scalar.copy(st[:mp], ps[:mp])
                        sq = sqp.tile([128, NT], F32, tag="sq")
                        nc.vector.tensor_mul(sq[:mp], st[:mp], st[:mp])
                        nc.vector.reduce_sum(parts[mi][:mp, t:t + 1], sq[:mp], axis=X)
                        nc.sync.dma_start(qk[m0:m0 + mp, t * NT:(t + 1) * NT], st[:mp])
                if variant != "ab":
                    if t >= 1:
                        issue_logits(t - 1)
            if variant == "ab":
                return
            issue_logits(NTB - 1)

        # -------- Phase C (small): norms, softmax, FW --------
        with (tc.tile_pool(name="small", bufs=1) as smallp,
              tc.tile_pool(name="soft", bufs=4) as softp,
              tc.tile_pool(name="psF", bufs=2, space="PSUM") as psF):
            wpt_sb = smallp.tile([CH, HEADS * C], BF, tag="wpt")
            nc.sync.dma_start(wpt_sb[:], wpt_d[:, :])
            scs = smallp.tile([CH, HEADS], F32, tag="scs")
            nc.gpsimd.dma_start(out=scs[:],
                                in_=scale_d[0:1, :].to_broadcast((CH, HEADS)))

            for mi, (m0, mp) in enumerate(KT[:3]):
                ssq = smallp.tile([128, 1], F32, tag=f"ssq{mi}", name=f"ssq{mi}")
                nc.vector.reduce_sum(ssq[:mp], parts[mi][:mp, :], axis=X)
                nc.scalar.sqrt(ssq[:mp], ssq[:mp])
                nc.vector.reciprocal(ssq[:mp], ssq[:mp])
                nc.sync.dma_start(rinv_d[0, 128 * mi:128 * mi + mp], ssq[:mp, 0])

            alpha = smallp.tile([CH, HEADS], F32, tag="alpha")
            for h in range(HEADS):
                nc.sync.dma_start(alpha[:, h:h + 1], rinv_d[0, CH * h:CH * (h + 1)])
            nc.vector.tensor_mul(alpha[:], alpha[:], scs[:])
            ball = smallp.tile([CH, C], F32, tag="ball")
            nc.gpsimd.dma_start(out=ball[:],
                                in_=rinv_d[0:1, C:2 * C].to_broadcast((CH, C)))

            for h in range(HEADS):
                gh = Gall[:, CH * h:CH * (h + 1)]
                nc.vector.tensor_scalar_mul(gh, gh, alpha[:, h:h + 1])
                gsb = softp.tile([CH, CH], F32, tag="gsb")
                nc.vector.tensor_mul(gsb[:], gh, ball[:, CH * h:CH * (h + 1)])
                mx = softp.tile([CH, 1], F32, tag="mx")
                nc.vector.reduce_max(mx[:], gsb[:], axis=X)
                nc.vector.tensor_scalar_mul(mx[:], mx[:], -1.0)
                ex = softp.tile([CH, CH], F32, tag="ex")
                nc.scalar.activation(ex[:], gsb[:],
                                     mybir.ActivationFunctionType.Exp,
                                     bias=mx[:], scale=1.0)
                sm = softp.tile([CH, 1], F32, tag="sm")
                nc.vector.reduce_sum(sm[:], ex[:], axis=X)
                nc.vector.reciprocal(sm[:], sm[:])
                asb = softp.tile([CH, CH], BF, tag="asb")
                nc.vector.tensor_scalar_mul(asb[:], ex[:], sm[:, 0:1])
                fw_ps = psF.tile([CH, C], F32, tag="fw")
                nc.tensor.matmul(fw_ps[:], asb[:], wpt_sb[:, C * h:C * (h + 1)],
                                 start=True, stop=True)
                fw_sb = softp.tile([CH, C], BF, tag="fwsb")
                nc.any.tensor_copy(fw_sb[:], fw_ps[:])
                nc.sync.dma_start(fwt_d[CH * h:CH * (h + 1), :], fw_sb[:])

            fwt0 = smallp.tile([128, C], BF, tag="fwt0")
            nc.sync.dma_start(fwt0[:], fwt_d[0:128, :])
            fwt1 = smallp.tile([64, C], BF, tag="fwt1")
            nc.sync.dma_start(fwt1[:], fwt_d[128:C, :])

            # -------- Phase D: out = FW @ v --------
            with (tc.tile_pool(name="ostage", bufs=6) as ostagep,
                  tc.tile_pool(name="psD", bufs=4, space="PSUM") as psD):
                for t in range(NTB):
                    for oi, (m0, mp) in enumerate(MT_OUT):
                        ps = psD.tile([128, NT], F32, tag="psD")
                        nc.tensor.matmul(ps[:mp], fwt0[:, m0:m0 + mp],
                                         v0[:, t * NT:(t + 1) * NT],
                                         start=True, stop=False)
                        nc.tensor.matmul(ps[:mp], fwt1[:, m0:m0 + mp],
                                         v1[:, t * NT:(t + 1) * NT],
                                         start=False, stop=True)
                        ost = ostagep.tile([128, NT], F32, tag="ost")
                        if (t + oi) % 2 == 0:
                            nc.vector.tensor_copy(ost[:mp], ps[:mp])
                        else:
                            nc.scalar.copy(ost[:mp], ps[:mp])
                        nc.sync.dma_start(out_d[m0:m0 + mp, t * NT:(t + 1) * NT],
                                          ost[:mp])
        psG_ctx.__exit__(None, None, None)


def _prep_shared(w_qkv1, w_qkv2, w_proj, scale):
    w1t = np.zeros((CP2, C3), dtype=BF16NP)
    w1t[:C] = np.ascontiguousarray(w_qkv1[:, :, 0, 0].T).astype(BF16NP)
    w2t = np.transpose(w_qkv2, (2, 3, 1, 0)).reshape(9, C3, C3)          # [s,i,o]

    # bf16 weights for v output channels (384:576): [5, 128, 9*192]
    w2v = np.zeros((5, 128, 9 * CV), dtype=BF16NP)
    for kt, (k0, kp) in enumerate(KT):
        w2v[kt, :kp, :] = np.ascontiguousarray(
            np.transpose(w2t[:, k0:k0 + kp, CQK:], (1, 0, 2)).reshape(kp, 9 * CV)
        ).astype(BF16NP)

    # fp8 weights for q/k output channels (0:384), scaled x64.
    w2qk = w2t[:, :, :CQK] * W8SCALE                                      # [9,576,384]
    # DoubleRow pairs: pair pp covers input subtiles (2pp, 2pp+1)
    w8p = np.zeros((2, 128, 2, 9 * CQK), dtype=F8NP)
    for pp in range(2):
        for j in range(2):
            k0 = 128 * (2 * pp + j)
            w8p[pp, :, j, :] = np.ascontiguousarray(
                np.transpose(w2qk[:, k0:k0 + 128, :], (1, 0, 2)).reshape(128, 9 * CQK)
            ).astype(F8NP)
    # plain-fp8 subtile 4 (input rows 512:576, zero-padded to 128)
    w8s = np.zeros((128, 9 * CQK), dtype=F8NP)
    w8s[:64, :] = np.ascontiguousarray(
        np.transpose(w2qk[:, 512:576, :], (1, 0, 2)).reshape(64, 9 * CQK)
    ).astype(F8NP)

    wpf = w_proj[:, :, 0, 0].T                                            # [c,o]
    wpt = np.concatenate([wpf[h * CH:(h + 1) * CH, :] for h in range(HEADS)],
                         axis=1).astype(BF16NP)                           # [48,768]
    sc = np.asarray(scale, np.float32).reshape(1, HEADS)
    return w1t, w2v, w8p, w8s, wpt, sc


def _make_in_maps(x, w_qkv1, w_qkv2, w_proj, scale):
    w1t, w2v, w8p, w8s, wpt, sc = _prep_shared(
        np.asarray(w_qkv1, np.float32), np.asarray(w_qkv2, np.float32),
        np.asarray(w_proj, np.float32), np.asarray(scale, np.float32))
    x = np.asarray(x, np.float32)
    xp = np.zeros((B, CP2, HP, WP), np.float32)
    xp[:, :C, 1:H + 1, 1:W + 1] = x
    xp = xp.astype(BF16NP).reshape(B, CP2, NPP)
    return [{"xp": xp[i], "w1t": w1t, "w2v": w2v,
             "w8p": w8p.reshape(2, 128, 2 * 9 * CQK), "w8s": w8s,
             "wpt": wpt, "scale": sc}
            for i in range(B)]


def kernel(x, w_qkv1, w_qkv2, w_proj, scale):
    if "nc" not in _CACHE:
        _CACHE["nc"] = _build()
    nc = _CACHE["nc"]
    in_maps = _make_in_maps(x, w_qkv1, w_qkv2, w_proj, scale)
    res = run_bass_kernel_spmd(nc, in_maps, core_ids=list(range(B)))
    out = np.stack([res.results[i]["out"].reshape(C, H, W) for i in range(B)], 0)
    return np.ascontiguousarray(out.astype(np.float32))


# revision 12
# speedup vs baseline: 1.4515x; 1.1678x over previous
"""Trainium2 Bass kernel for channel ("transposed") attention:
  qkv = conv3x3(conv1x1(x)); per-head L2-normalized channel attention; 1x1 proj.

Sharding: pure data-parallel — batch 8 across 8 NeuronCores (one image each).
Per-core pipeline (matmuls accumulate in f32 PSUM):
  A+B fused: conv1x1 chunks stream into a circular SBUF row buffer (19 rows
     x 130 px per input subtile, 16-row period + 3 halo rows) — no DRAM
     round-trip for y1. conv3x3 consumes 6-row windows:
       q/k output channels (0:384): fp8e4 DoubleRow matmuls (2 subtile
       pairs + 1 plain fp8 K=64 matmul per shift; weights host-scaled x64
       to dodge e4m3 subnormals — the scale cancels in the L2 norms),
       v channels (384:576): bf16; v0 SBUF-resident; the 64-wide v1 tile
       runs as column-tiled concurrent matmul pairs across two n-tiles.
     q/k streamed to DRAM (1024-wide stores) + squared-norm reduce;
     per-n-tile qk DMA-transposes and per-head logit matmuls interleaved
     (one tile lagged) so the PE stream never breaks.
  C: norms, softmax, FW = wproj @ blockdiag(attn) — all-SBUF plumbing,
     tiny keep-warm matmuls so the PE clock doesn't re-throttle.
  D: out = FW @ v -> f32 output.
"""
import numpy as np
import ml_dtypes

import concourse.bass as bass
import concourse.tile as tile
from concourse import bacc, mybir
from concourse.bass_utils import run_bass_kernel_spmd

BF16NP = ml_dtypes.bfloat16
F8NP = ml_dtypes.float8_e4m3
BF = mybir.dt.bfloat16
F8 = mybir.dt.float8e4
F32 = mybir.dt.float32
DR = mybir.MatmulPerfMode.DoubleRow

B, C, H, W = 8, 192, 128, 128
HEADS, CH = 4, 48
C3 = 3 * C                      # 576
CP2 = 256                       # 192 padded to 2*128
HP, WP = H + 2, W + 2           # 130
NPIX = H * W                    # 16384
NPP = HP * WP                   # 16900
NT = 512
NTB = NPIX // NT                # 32 conv n-tiles
KT = [(0, 128), (128, 128), (256, 128), (384, 128), (512, 64)]   # 576 split
MT_OUT = [(0, 128), (128, 64)]                                   # out-ch split
CQK = 384                       # q/k output channels
CV = 192                        # v output channels
W8SCALE = 64.0                  # fp8 q/k weight prescale (cancels in L2 norm)

RBP = 16                        # circular buffer period (rows)
RBR = RBP + 3                   # +3 halo rows (dup of rows 0..2)
RB = RBP * WP                   # 2080 px period
HALO = 3 * WP                   # 390 px halo

# phase-A chunks: 33 at 512*c covering [0,16896), plus one at NPP-512
A_OFFS = [512 * c for c in range(33)] + [NPP - NT]
NCH = len(A_OFFS)               # 34


def _chunk_need(t):
    """chunks that must be issued before conv tile t (pixels < 520*t+780)."""
    if t >= NTB:
        return NCH
    need = 0
    cover = 520 * t + 780
    for c, off in enumerate(A_OFFS):
        if off < cover:
            need = c + 1
    return need


def _chunk_conflicts(c, consumed):
    """True if chunk c's circular-buffer write would clobber a window still
    needed by a tile whose reads haven't been issued yet (> consumed).
    Chunk c evicts the pixels one buffer-lap back: [off-RB, off-RB+NT)."""
    lo, hi = A_OFFS[c] - RB, A_OFFS[c] - RB + NT
    for tp in range(max(consumed + 1, 0), NTB):
        w0 = 520 * tp
        if w0 < hi and lo < w0 + 780:
            return True
    return False


_CACHE = {}


def _build():
    nc = bacc.Bacc("TRN2", target_bir_lowering=False, debug=False, num_devices=8)
    xp_d = nc.dram_tensor("xp", [CP2, NPP], BF, kind="ExternalInput").ap()
    w1t_d = nc.dram_tensor("w1t", [CP2, C3], BF, kind="ExternalInput").ap()
    w2v_d = nc.dram_tensor("w2v", [5, 128, 9 * CV], BF, kind="ExternalInput").ap()
    w8p_d = nc.dram_tensor("w8p", [2, 128, 2 * 9 * CQK], F8, kind="ExternalInput").ap()
    w8s_d = nc.dram_tensor("w8s", [64, 9 * CQK], F8, kind="ExternalInput").ap()
    wpt_d = nc.dram_tensor("wpt", [CH, HEADS * C], BF, kind="ExternalInput").ap()
    scale_d = nc.dram_tensor("scale", [1, HEADS], F32, kind="ExternalInput").ap()
    out_d = nc.dram_tensor("out", [C, NPIX], F32, kind="ExternalOutput").ap()

    with tile.TileContext(nc) as tc:
        with tc.tile_pool(name="dram", bufs=1, space="DRAM") as dram:
            qk = dram.tile([2 * C, NPIX], BF)
            rinv_d = dram.tile([1, 512], F32)
            _build_body(nc, tc, xp_d, w1t_d, w2v_d, w8p_d, w8s_d, wpt_d,
                        scale_d, out_d, qk, rinv_d)
    nc.compile()
    return nc


def _build_body(nc, tc, xp_d, w1t_d, w2v_d, w8p_d, w8s_d, wpt_d, scale_d,
                out_d, qk, rinv_d):
    X = mybir.AxisListType.X

    with tc.tile_pool(name="persist", bufs=1) as persist:
        v0 = persist.tile([128, NPIX], BF, tag="v0")
        # v1 (64 ch): even tiles on partitions 0:64, odd on 64:128
        v1t = persist.tile([128, NPIX // 2], BF, tag="v1t")
        parts = [persist.tile([mp, NTB], F32, tag=f"part{i}", name=f"part{i}")
                 for i, (m0, mp) in enumerate(KT[:3])]
        # circular y1 row buffers, one per input subtile
        y1b = [persist.tile([128 if i < 4 else 64, RBR * WP], BF,
                            tag=f"y1b{i}", name=f"y1b{i}") for i in range(5)]
        y1i = [b.rearrange("p (r w) -> p r w", r=RBR) for b in y1b]
        psG_ctx = tc.tile_pool(name="psG", bufs=1, space="PSUM")
        psG = psG_ctx.__enter__()

        with (tc.tile_pool(name="wts", bufs=1) as wts,
              tc.tile_pool(name="xk", bufs=8) as xkp,
              tc.tile_pool(name="slab8", bufs=3) as slab8p,
              tc.tile_pool(name="stage", bufs=6) as stagep,
              tc.tile_pool(name="sq", bufs=3) as sqp,
              tc.tile_pool(name="qkt", bufs=8) as qktp,
              tc.tile_pool(name="psA", bufs=5, space="PSUM") as psA,
              tc.tile_pool(name="psV", bufs=2, space="PSUM") as psV):

            w1s = []
            for i in range(2):
                t = wts.tile([128, C3], BF, tag=f"w1_{i}", name=f"w1_{i}")
                nc.sync.dma_start(t[:], w1t_d[128 * i:128 * (i + 1), :])
                w1s.append(t)
            w2vs = [wts.tile([128 if i < 4 else 64, 9 * CV], BF,
                             tag=f"w2v_{i}", name=f"w2v_{i}") for i in range(5)]
            w8ps = [wts.tile([128, 2, 9 * CQK], F8, tag=f"w8p_{i}", name=f"w8p_{i}")
                    for i in range(2)]
            w8ss = wts.tile([64, 9 * CQK], F8, tag="w8s", name="w8s")

            def load_phaseB_weights():
                for i in range(5):
                    kp = 128 if i < 4 else 64
                    nc.sync.dma_start(w2vs[i][:], w2v_d[i, :kp, :])
                for i in range(2):
                    nc.sync.dma_start(w8ps[i][:], w8p_d[i, :, :])
                nc.sync.dma_start(w8ss[:], w8s_d[:, :])

            # ---- phase-A chunk issuer: y1 = w1 @ xp into circular buffer ----
            xk_big = {}

            def issue_chunk(c):
                off = A_OFFS[c]
                if c < 32:
                    big = c // 2
                    if big not in xk_big:
                        boff = 1024 * big
                        xs = []
                        for i in range(2):
                            xk = xkp.tile([128, 1024], BF, tag="xk")
                            nc.sync.dma_start(
                                xk[:], xp_d[128 * i:128 * (i + 1), boff:boff + 1024])
                            xs.append(xk)
                        xk_big[big] = xs
                        if len(xk_big) > 3:
                            del xk_big[min(xk_big)]
                    xs = xk_big[big]
                    half = (c % 2) * 512
                    mov = [x[:, half:half + 512] for x in xs]
                else:
                    xs = []
                    for i in range(2):
                        xk = xkp.tile([128, NT], BF, tag="xks", name="xks")
                        nc.sync.dma_start(xk[:], xp_d[128 * i:128 * (i + 1),
                                                      off:off + NT])
                        xs.append(xk)
                    mov = [x[:] for x in xs]

                # circular positions + halo duplication pieces
                pos = off % RB
                pieces = []
                for s0, s1 in ([(0, RB - pos), (RB - pos, NT)]
                               if pos + NT > RB else [(0, NT)]):
                    p0 = (pos + s0) % RB
                    pieces.append((p0, s0, s1))
                    if p0 < HALO:        # duplicate into halo rows
                        e = min(HALO - p0, s1 - s0)
                        pieces.append((p0 + RB, s0, s0 + e))

                for mi, (m0, mp) in enumerate(KT):
                    ps = psA.tile([128, NT], F32, tag="ps")
                    for i in range(2):
                        nc.tensor.matmul(ps[:mp], w1s[i][:, m0:m0 + mp],
                                         mov[i], start=(i == 0), stop=(i == 1))
                    for pi, (p0, s0, s1) in enumerate(pieces):
                        if (mi + pi) % 2 == 0:
                            nc.vector.tensor_copy(
                                y1b[mi][:mp, p0:p0 + (s1 - s0)], ps[:mp, s0:s1])
                        else:
                            nc.scalar.copy(
                                y1b[mi][:mp, p0:p0 + (s1 - s0)], ps[:mp, s0:s1])

            # logits PSUM accumulator: 4 heads packed in one bank [48, 192]
            Gall = psG.tile([CH, HEADS * CH], F32, tag="Gall")

            def issue_logits(tt):
                for j in range(4 * tt, 4 * tt + 4):
                    qkt = qktp.tile([128, 2 * C], BF, tag="qkt", name="qkt")
                    nc.sync.dma_start_transpose(qkt[:], qk[:, j * 128:(j + 1) * 128])
                    for h in range(HEADS):
                        nc.tensor.matmul(
                            Gall[:, CH * h:CH * (h + 1)],
                            qkt[:, CH * h:CH * (h + 1)],
                            qkt[:, C + CH * h:C + CH * (h + 1)],
                            start=(j == 0 and h == 0),
                            stop=(j == 127 and h == HEADS - 1),
                            skip_group_check=True)

            # ---- fused pipeline ----
            for c in range(_chunk_need(1)):
                issue_chunk(c)
            load_phaseB_weights()
            issued = _chunk_need(1)
            stq = [None] * 3
            ps_v1 = [None]

            for t in range(NTB):
                r6 = (4 * t) % RBP
                # fp8 conversion of the 6-row window, all 5 subtiles
                s8 = slab8p.tile([128, 5, 6, WP], F8, tag="s8")
                for i in range(5):
                    kp = 128 if i < 4 else 64
                    if i % 2 == 0:
                        nc.vector.tensor_copy(s8[:kp, i], y1i[i][:, r6:r6 + 6, :])
                    else:
                        nc.scalar.copy(s8[:kp, i], y1i[i][:, r6:r6 + 6, :])

                for mi, (m0, mp) in enumerate(KT[:4]):
                    ps = psA.tile([128, NT], F32, tag="ps")
                    if mi < 3:      # q/k: fp8 DoubleRow
                        n_mm = 0
                        for s in range(9):
                            dy, dx = s // 3, s % 3
                            for pp in range(2):
                                nc.tensor.matmul(
                                    ps[:mp],
                                    w8ps[pp][:, :, s * CQK + m0: s * CQK + m0 + mp],
                                    s8[:, 2 * pp:2 * pp + 2, dy:dy + 4, dx:dx + W],
                                    start=(n_mm == 0), stop=False,
                                    perf_mode=DR)
                                n_mm += 1
                            nc.tensor.matmul(
                                ps[:mp],
                                w8ss[:, s * CQK + m0: s * CQK + m0 + mp],
                                s8[:64, 4, dy:dy + 4, dx:dx + W],
                                start=False, stop=(s == 8))
                        # stage 2 tiles into one 1024-wide store
                        half = (t % 2) * NT
                        if t % 2 == 0:
                            stq[mi] = stagep.tile([128, 2 * NT], BF, tag="stage", name=f"stq{mi}")
                        st = stq[mi]
                        if mi % 2 == 0:
                            nc.vector.tensor_copy(st[:mp, half:half + NT], ps[:mp])
                        else:
                            nc.scalar.copy(st[:mp, half:half + NT], ps[:mp])
                        sq = sqp.tile([128, NT], F32, tag="sq")
                        nc.vector.tensor_mul(sq[:mp], st[:mp, half:half + NT],
                                             st[:mp, half:half + NT])
                        nc.vector.reduce_sum(parts[mi][:mp, t:t + 1], sq[:mp], axis=X)
                        if t % 2 == 1:
                            nc.sync.dma_start(
                                qk[m0:m0 + mp, (t - 1) * NT:(t + 1) * NT], st[:mp])
                    else:           # v0: bf16
                        n_mm = 0
                        for s in range(9):
                            dy, dx = s // 3, s % 3
                            r = (4 * t + dy) % RBP
                            for i in range(5):
                                kp = 128 if i < 4 else 64
                                nc.tensor.matmul(
                                    ps[:mp],
                                    w2vs[i][:, s * CV + m0 - CQK:
                                            s * CV + m0 - CQK + mp],
                                    y1i[i][:, r:r + 4, dx:dx + W],
                                    start=(n_mm == 0), stop=(n_mm == 44))
                                n_mm += 1
                        nc.scalar.copy(v0[:, t * NT:(t + 1) * NT], ps[:])

                # v1 (64 ch): column-tiled concurrent pairs over (t-1, t)
                if t % 2 == 0:
                    ps_v1[0] = psV.tile([128, NT], F32, tag="psv1", name="psv1")
                else:
                    psv = ps_v1[0]
                    n_mm = 0
                    for s in range(9):
                        dy, dx = s // 3, s % 3
                        ra = (4 * (t - 1) + dy) % RBP
                        rb = (4 * t + dy) % RBP
                        for i in range(5):
                            wsl = w2vs[i][:, s * CV + 128: s * CV + 192]
                            nc.tensor.matmul(
                                psv[0:64], wsl, y1i[i][:, ra:ra + 4, dx:dx + W],
                                start=(n_mm == 0), stop=(n_mm == 44),
                                skip_group_check=True)
                            nc.tensor.matmul(
                                psv[64:128], wsl, y1i[i][:, rb:rb + 4, dx:dx + W],
                                start=(n_mm == 0), stop=(n_mm == 44),
                                skip_group_check=True)
                            n_mm += 1
                    cols = (t // 2) * NT
                    nc.vector.tensor_copy(v1t[0:64, cols:cols + NT], psv[0:64])
                    nc.scalar.copy(v1t[64:128, cols:cols + NT], psv[64:128])

                if t >= 1:
                    issue_logits(t - 1)
                # tiles fully read-issued: t's v1 window is still pending
                # until the odd-t pair fires
                consumed = t - 1 if t % 2 == 0 else t
                while (issued < NCH and issued < _chunk_need(t + 2)
                       and not _chunk_conflicts(issued, consumed)):
                    issue_chunk(issued)
                    issued += 1
                assert issued >= _chunk_need(t + 1), (t, issued)
            assert issued == NCH
            issue_logits(NTB - 1)

        # -------- Phase C (small): norms, softmax, FW --------
        with (tc.tile_pool(name="small", bufs=1) as smallp,
              tc.tile_pool(name="soft", bufs=4) as softp,
              tc.tile_pool(name="psF", bufs=1, space="PSUM") as psF):
            wpt_sb = smallp.tile([CH, HEADS * C], BF, tag="wpt")
            nc.sync.dma_start(wpt_sb[:], wpt_d[:, :])
            scs = smallp.tile([CH, HEADS], F32, tag="scs")
            nc.gpsimd.dma_start(out=scs[:],
                                in_=scale_d[0:1, :].to_broadcast((CH, HEADS)))

            for mi, (m0, mp) in enumerate(KT[:3]):
                ssq = smallp.tile([128, 1], F32, tag=f"ssq{mi}", name=f"ssq{mi}")
                nc.vector.reduce_sum(ssq[:mp], parts[mi][:mp, :], axis=X)
                nc.scalar.sqrt(ssq[:mp], ssq[:mp])
                nc.vector.reciprocal(ssq[:mp], ssq[:mp])
                nc.sync.dma_start(rinv_d[0, 128 * mi:128 * mi + mp], ssq[:mp, 0])
                # keep-warm: tiny matmul dependent on ssq
                kw = psF.tile([1, NTB], F32, tag="kw", name=f"kw{mi}")
                nc.tensor.matmul(kw[:], ssq[:mp, 0:1], parts[mi][:mp, :],
                                 start=True, stop=True)

            alpha = smallp.tile([CH, HEADS], F32, tag="alpha")
            for h in range(HEADS):
                nc.sync.dma_start(alpha[:, h:h + 1], rinv_d[0, CH * h:CH * (h + 1)])
            nc.vector.tensor_mul(alpha[:], alpha[:], scs[:])
            ball = smallp.tile([CH, C], F32, tag="ball")
            nc.gpsimd.dma_start(out=ball[:],
                                in_=rinv_d[0:1, C:2 * C].to_broadcast((CH, C)))

            fwt0 = smallp.tile([128, C], BF, tag="fwt0")
            fwt1 = smallp.tile([128, C], BF, tag="fwt1")
            for h in range(HEADS):
                gh = Gall[:, CH * h:CH * (h + 1)]
                nc.vector.tensor_scalar_mul(gh, gh, alpha[:, h:h + 1])
                gsb = softp.tile([CH, CH], F32, tag="gsb")
                nc.vector.tensor_mul(gsb[:], gh, ball[:, CH * h:CH * (h + 1)])
                mx = softp.tile([CH, 1], F32, tag="mx")
                nc.vector.reduce_max(mx[:], gsb[:], axis=X)
                nc.vector.tensor_scalar_mul(mx[:], mx[:], -1.0)
                ex = softp.tile([CH, CH], F32, tag="ex")
                nc.scalar.activation(ex[:], gsb[:],
                                     mybir.ActivationFunctionType.Exp,
                                     bias=mx[:], scale=1.0)
                sm = softp.tile([CH, 1], F32, tag="sm")
                nc.vector.reduce_sum(sm[:], ex[:], axis=X)
                nc.vector.reciprocal(sm[:], sm[:])
                asb = softp.tile([CH, CH], BF, tag="asb")
                nc.vector.tensor_scalar_mul(asb[:], ex[:], sm[:, 0:1])
                fw_ps = psF.tile([CH, C], F32, tag="fw")
                nc.tensor.matmul(fw_ps[:], asb[:], wpt_sb[:, C * h:C * (h + 1)],
                                 start=True, stop=True)
                fw_sb = softp.tile([CH, C], BF, tag="fwsb")
                nc.any.tensor_copy(fw_sb[:], fw_ps[:])
                # scatter into fwt0 (rows 0:128) / fwt1 (rows 128:192, duplicated
                # at partitions 0:64 and 64:128 for the column-tiled v1 layout)
                r0 = CH * h
                n0 = min(CH, max(0, 128 - r0))
                if n0 > 0:
                    nc.sync.dma_start(fwt0[r0:r0 + n0, :], fw_sb[0:n0, :])
                if n0 < CH:
                    r1 = r0 + n0 - 128
                    nc.sync.dma_start(fwt1[r1:r1 + CH - n0, :], fw_sb[n0:CH, :])
                    nc.sync.dma_start(fwt1[64 + r1:64 + r1 + CH - n0, :],
                                      fw_sb[n0:CH, :])

            # -------- Phase D: out = FW @ v --------
            with (tc.tile_pool(name="ostage", bufs=6) as ostagep,
                  tc.tile_pool(name="psD", bufs=4, space="PSUM") as psD):
                for t in range(NTB):
                    vb = 64 * (t % 2)
                    vcols = (t // 2) * NT
                    for oi, (m0, mp) in enumerate(MT_OUT):
                        ps = psD.tile([128, NT], F32, tag="psD")
                        nc.tensor.matmul(ps[:mp], fwt0[:, m0:m0 + mp],
                                         v0[:, t * NT:(t + 1) * NT],
                                         start=True, stop=False)
                        nc.tensor.matmul(ps[:mp], fwt1[vb:vb + 64, m0:m0 + mp],
                                         v1t[vb:vb + 64, vcols:vcols + NT],
                                         start=False, stop=True)
                        ost = ostagep.tile([128, NT], F32, tag="ost")
                        if (t + oi) % 2 == 0:
                            nc.vector.tensor_copy(ost[:mp], ps[:mp])
                        else:
                            nc.scalar.copy(ost[:mp], ps[:mp])
                        nc.sync.dma_start(out_d[m0:m0 + mp, t * NT:(t + 1) * NT],
                                          ost[:mp])
        psG_ctx.__exit__(None, None, None)


def _prep_shared(w_qkv1, w_qkv2, w_proj, scale):
    w1t = np.zeros((CP2, C3), dtype=BF16NP)
    w1t[:C] = np.ascontiguousarray(w_qkv1[:, :, 0, 0].T).astype(BF16NP)
    w2t = np.transpose(w_qkv2, (2, 3, 1, 0)).reshape(9, C3, C3)          # [s,i,o]

    # bf16 weights for v output channels (384:576): [5, 128, 9*192]
    w2v = np.zeros((5, 128, 9 * CV), dtype=BF16NP)
    for kt, (k0, kp) in enumerate(KT):
        w2v[kt, :kp, :] = np.ascontiguousarray(
            np.transpose(w2t[:, k0:k0 + kp, CQK:], (1, 0, 2)).reshape(kp, 9 * CV)
        ).astype(BF16NP)

    # fp8 weights for q/k output channels (0:384), scaled x64.
    w2qk = w2t[:, :, :CQK] * W8SCALE                                      # [9,576,384]
    w8p = np.zeros((2, 128, 2, 9 * CQK), dtype=F8NP)
    for pp in range(2):
        for j in range(2):
            k0 = 128 * (2 * pp + j)
            w8p[pp, :, j, :] = np.ascontiguousarray(
                np.transpose(w2qk[:, k0:k0 + 128, :], (1, 0, 2)).reshape(128, 9 * CQK)
            ).astype(F8NP)
    w8s = np.ascontiguousarray(
        np.transpose(w2qk[:, 512:576, :], (1, 0, 2)).reshape(64, 9 * CQK)
    ).astype(F8NP)

    wpf = w_proj[:, :, 0, 0].T                                            # [c,o]
    wpt = np.concatenate([wpf[h * CH:(h + 1) * CH, :] for h in range(HEADS)],
                         axis=1).astype(BF16NP)                           # [48,768]
    sc = np.asarray(scale, np.float32).reshape(1, HEADS)
    return w1t, w2v, w8p, w8s, wpt, sc


def _make_in_maps(x, w_qkv1, w_qkv2, w_proj, scale):
    w1t, w2v, w8p, w8s, wpt, sc = _prep_shared(
        np.asarray(w_qkv1, np.float32), np.asarray(w_qkv2, np.float32),
        np.asarray(w_proj, np.float32), np.asarray(scale, np.float32))
    x = np.asarray(x, np.float32)
    xp = np.zeros((B, CP2, HP, WP), np.float32)
    xp[:, :C, 1:H + 1, 1:W + 1] = x
    xp = xp.astype(BF16NP).reshape(B, CP2, NPP)
    return [{"xp": xp[i], "w1t": w1t, "w2v": w2v,
             "w8p": w8p.reshape(2, 128, 2 * 9 * CQK), "w8s": w8s,
             "wpt": wpt, "scale": sc}
            for i in range(B)]


def kernel(x, w_qkv1, w_qkv2, w_proj, scale):
    if "nc" not in _CACHE:
        _CACHE["nc"] = _build()
    nc = _CACHE["nc"]
    in_maps = _make_in_maps(x, w_qkv1, w_qkv2, w_proj, scale)
    res = run_bass_kernel_spmd(nc, in_maps, core_ids=list(range(B)))
    out = np.stack([res.results[i]["out"].reshape(C, H, W) for i in range(B)], 0)
    return np.ascontiguousarray(out.astype(np.float32))


# revision 14
# speedup vs baseline: 1.7706x; 1.2198x over previous
"""Trainium2 Bass kernel for channel ("transposed") attention:
  qkv = conv3x3(conv1x1(x)); per-head L2-normalized channel attention; 1x1 proj.

Sharding: pure data-parallel — batch 8 across 8 NeuronCores (one image each).
Per-core pipeline (matmuls accumulate in f32 PSUM):
  A+B fused: conv1x1 chunks stream into a circular SBUF row buffer (19 rows
     x 130 px per input subtile, 16-row period + 3 halo rows) — no DRAM
     round-trip for y1. conv3x3 consumes 6-row windows:
       q/k output channels (0:384): fp8e4 DoubleRow matmuls (2 subtile
       pairs + 1 plain fp8 matmul per shift; weights host-scaled x64
       to dodge e4m3 subnormals — the scale cancels in the L2 norms),
       v channels (384:576): bf16; v0 SBUF-resident; the 64-wide v1 tile
       runs as column-tiled concurrent matmul pairs across two n-tiles.
     q/k streamed to DRAM (1024-wide stores) + squared-norm reduce;
     per-n-tile qk DMA-transposes and per-head logit matmuls interleaved
     (one tile lagged) so the PE stream never breaks.
  C: norms, softmax, FW = wproj @ blockdiag(attn) — all-SBUF plumbing,
     tiny keep-warm matmuls so the PE clock doesn't re-throttle.
  D: out = FW @ v -> bf16 output (host upcasts to f32).
"""
import numpy as np
import ml_dtypes

import concourse.bass as bass
import concourse.tile as tile
from concourse import bacc, mybir
from concourse.bass_utils import run_bass_kernel_spmd

BF16NP = ml_dtypes.bfloat16
F8NP = ml_dtypes.float8_e4m3
BF = mybir.dt.bfloat16
F8 = mybir.dt.float8e4
F32 = mybir.dt.float32
DR = mybir.MatmulPerfMode.DoubleRow

B, C, H, W = 8, 192, 128, 128
HEADS, CH = 4, 48
C3 = 3 * C                      # 576
CP2 = 256                       # 192 padded to 2*128
HP, WP = H + 2, W + 2           # 130
NPIX = H * W                    # 16384
NPP = HP * WP                   # 16900
NT = 512
NTB = NPIX // NT                # 32 conv n-tiles
KT = [(0, 128), (128, 128), (256, 128), (384, 128), (512, 64)]   # 576 split
MT_OUT = [(0, 128), (128, 64)]                                   # out-ch split
CQK = 384                       # q/k output channels
CV = 192                        # v output channels
W8SCALE = 64.0                  # fp8 q/k weight prescale (cancels in L2 norm)

RBP = 16                        # circular buffer period (rows)
RBR = RBP + 3                   # +3 halo rows (dup of rows 0..2)
RB = RBP * WP                   # 2080 px period
HALO = 3 * WP                   # 390 px halo

# phase-A chunks: 33 at 512*c covering [0,16896), plus one at NPP-512
A_OFFS = [512 * c for c in range(33)] + [NPP - NT]
NCH = len(A_OFFS)               # 34


def _chunk_need(t):
    """chunks that must be issued before conv tile t (pixels < 520*t+780)."""
    if t >= NTB:
        return NCH
    need = 0
    cover = 520 * t + 780
    for c, off in enumerate(A_OFFS):
        if off < cover:
            need = c + 1
    return need


def _chunk_conflicts(c, consumed):
    """True if chunk c's circular-buffer write would clobber a window still
    needed by a tile whose reads haven't been issued yet (> consumed).
    Chunk c evicts the pixels one buffer-lap back: [off-RB, off-RB+NT)."""
    lo, hi = A_OFFS[c] - RB, A_OFFS[c] - RB + NT
    for tp in range(max(consumed + 1, 0), NTB):
        w0 = 520 * tp
        if w0 < hi and lo < w0 + 780:
            return True
    return False


_CACHE = {}


def _build():
    nc = bacc.Bacc("TRN2", target_bir_lowering=False, debug=False, num_devices=8)
    xp_d = nc.dram_tensor("xp", [CP2, NPP], BF, kind="ExternalInput").ap()
    w1t_d = nc.dram_tensor("w1t", [CP2, C3], BF, kind="ExternalInput").ap()
    w2v_d = nc.dram_tensor("w2v", [5, 128, 9 * CV], BF, kind="ExternalInput").ap()
    w8p_d = nc.dram_tensor("w8p", [2, 128, 2 * 9 * CQK], F8, kind="ExternalInput").ap()
    w8s_d = nc.dram_tensor("w8s", [128, 9 * CQK], F8, kind="ExternalInput").ap()
    wpt_d = nc.dram_tensor("wpt", [CH, HEADS * C], BF, kind="ExternalInput").ap()
    scale_d = nc.dram_tensor("scale", [1, HEADS], F32, kind="ExternalInput").ap()
    out_d = nc.dram_tensor("out", [C, NPIX], BF, kind="ExternalOutput").ap()

    with tile.TileContext(nc) as tc:
        with tc.tile_pool(name="dram", bufs=1, space="DRAM") as dram:
            qk = dram.tile([2 * C, NPIX], BF)
            rinv_d = dram.tile([1, 512], F32)
            _build_body(nc, tc, xp_d, w1t_d, w2v_d, w8p_d, w8s_d, wpt_d,
                        scale_d, out_d, qk, rinv_d)
    nc.compile()
    return nc


def _build_body(nc, tc, xp_d, w1t_d, w2v_d, w8p_d, w8s_d, wpt_d, scale_d,
                out_d, qk, rinv_d):
    X = mybir.AxisListType.X

    with tc.tile_pool(name="persist", bufs=1) as persist:
        v0 = persist.tile([128, NPIX], BF, tag="v0")
        # v1 (64 ch): even tiles on partitions 0:64, odd on 64:128
        v1t = persist.tile([128, NPIX // 2], BF, tag="v1t")
        parts = [persist.tile([mp, NTB], F32, tag=f"part{i}", name=f"part{i}")
                 for i, (m0, mp) in enumerate(KT[:3])]
        # circular y1 row buffers, one per input subtile
        y1b = [persist.tile([128, RBR * WP], BF,
                            tag=f"y1b{i}", name=f"y1b{i}") for i in range(5)]
        y1i = [b.rearrange("p (r w) -> p r w", r=RBR) for b in y1b]
        psG_ctx = tc.tile_pool(name="psG", bufs=1, space="PSUM")
        psG = psG_ctx.__enter__()

        with (tc.tile_pool(name="wts", bufs=1) as wts,
              tc.tile_pool(name="xk", bufs=8) as xkp,
              tc.tile_pool(name="slab8", bufs=3) as slab8p,
              tc.tile_pool(name="stage", bufs=6) as stagep,
              tc.tile_pool(name="sq", bufs=3) as sqp,
              tc.tile_pool(name="qkt", bufs=8) as qktp,
              tc.tile_pool(name="psA", bufs=5, space="PSUM") as psA,
              tc.tile_pool(name="psV", bufs=2, space="PSUM") as psV):

            w1s = []
            for i in range(2):
                t = wts.tile([128, C3], BF, tag=f"w1_{i}", name=f"w1_{i}")
                nc.sync.dma_start(t[:], w1t_d[128 * i:128 * (i + 1), :])
                w1s.append(t)
            w2vs = [wts.tile([128, 9 * CV], BF,
                             tag=f"w2v_{i}", name=f"w2v_{i}") for i in range(5)]
            w8ps = [wts.tile([128, 2, 9 * CQK], F8, tag=f"w8p_{i}", name=f"w8p_{i}")
                    for i in range(2)]
            w8ss = wts.tile([128, 9 * CQK], F8, tag="w8s", name="w8s")

            def load_phaseB_weights():
                for i in range(5):
                    nc.sync.dma_start(w2vs[i][:], w2v_d[i, :, :])
                for i in range(2):
                    nc.sync.dma_start(w8ps[i][:], w8p_d[i, :, :])
                nc.sync.dma_start(w8ss[:], w8s_d[:, :])

            # ---- phase-A chunk issuer: y1 = w1 @ xp into circular buffer ----
            xk_big = {}

            def issue_chunk(c):
                off = A_OFFS[c]
                if c < 32:
                    big = c // 2
                    if big not in xk_big:
                        boff = 1024 * big
                        xs = []
                        for i in range(2):
                            xk = xkp.tile([128, 1024], BF, tag="xk")
                            nc.sync.dma_start(
                                xk[:], xp_d[128 * i:128 * (i + 1), boff:boff + 1024])
                            xs.append(xk)
                        xk_big[big] = xs
                        if len(xk_big) > 3:
                            del xk_big[min(xk_big)]
                    xs = xk_big[big]
                    half = (c % 2) * 512
                    mov = [x[:, half:half + 512] for x in xs]
                else:
                    xs = []
                    for i in range(2):
                        xk = xkp.tile([128, NT], BF, tag="xks", name="xks")
                        nc.sync.dma_start(xk[:], xp_d[128 * i:128 * (i + 1),
                                                      off:off + NT])
                        xs.append(xk)
                    mov = [x[:] for x in xs]

                # circular positions + halo duplication pieces
                pos = off % RB
                pieces = []
                for s0, s1 in ([(0, RB - pos), (RB - pos, NT)]
                               if pos + NT > RB else [(0, NT)]):
                    p0 = (pos + s0) % RB
                    pieces.append((p0, s0, s1))
                    if p0 < HALO:        # duplicate into halo rows
                        e = min(HALO - p0, s1 - s0)
                        pieces.append((p0 + RB, s0, s0 + e))

                for mi, (m0, mp) in enumerate(KT):
                    ps = psA.tile([128, NT], F32, tag="ps")
                    for i in range(2):
                        nc.tensor.matmul(ps[:mp], w1s[i][:, m0:m0 + mp],
                                         mov[i], start=(i == 0), stop=(i == 1))
                    for pi, (p0, s0, s1) in enumerate(pieces):
                        if (mi + pi) % 2 == 0:
                            nc.vector.tensor_copy(
                                y1b[mi][:mp, p0:p0 + (s1 - s0)], ps[:mp, s0:s1])
                        else:
                            nc.scalar.copy(
                                y1b[mi][:mp, p0:p0 + (s1 - s0)], ps[:mp, s0:s1])

            # logits PSUM accumulator: 4 heads packed in one bank [48, 192]
            Gall = psG.tile([CH, HEADS * CH], F32, tag="Gall")

            def issue_logits(tt):
                for j in range(4 * tt, 4 * tt + 4):
                    qkt = qktp.tile([128, 2 * C], BF, tag="qkt", name="qkt")
                    nc.sync.dma_start_transpose(qkt[:], qk[:, j * 128:(j + 1) * 128])
                    for h in range(HEADS):
                        nc.tensor.matmul(
                            Gall[:, CH * h:CH * (h + 1)],
                            qkt[:, CH * h:CH * (h + 1)],
                            qkt[:, C + CH * h:C + CH * (h + 1)],
                            start=(j == 0 and h == 0),
                            stop=(j == 127 and h == HEADS - 1),
                            skip_group_check=True)

            # zero rows 64:128 of subtile-4 buffer once (K padding so every
            # matmul is a uniform K=128 — partial-K matmuls defeat the
            # LDWEIGHTS prefetch and cost ~100ns each)
            nc.vector.memset(y1b[4][64:128, :], 0.0)

            # ---- fused pipeline ----
            for c in range(_chunk_need(1)):
                issue_chunk(c)
            load_phaseB_weights()
            issued = _chunk_need(1)
            stq = [None] * 3
            ps_v1 = [None]

            for t in range(NTB):
                r6 = (4 * t) % RBP
                # fp8 conversion of the 6-row window, all 5 subtiles
                s8 = slab8p.tile([128, 5, 6, WP], F8, tag="s8")
                for i in range(5):
                    if i % 2 == 0:
                        nc.vector.tensor_copy(s8[:, i], y1i[i][:, r6:r6 + 6, :])
                    else:
                        nc.scalar.copy(s8[:, i], y1i[i][:, r6:r6 + 6, :])

                for mi, (m0, mp) in enumerate(KT[:4]):
                    ps = psA.tile([128, NT], F32, tag="ps")
                    if mi < 3:      # q/k: fp8 DoubleRow
                        n_mm = 0
                        for s in range(9):
                            dy, dx = s // 3, s % 3
                            for pp in range(2):
                                nc.tensor.matmul(
                                    ps[:mp],
                                    w8ps[pp][:, :, s * CQK + m0: s * CQK + m0 + mp],
                                    s8[:, 2 * pp:2 * pp + 2, dy:dy + 4, dx:dx + W],
                                    start=(n_mm == 0), stop=False,
                                    perf_mode=DR)
                                n_mm += 1
                            nc.tensor.matmul(
                                ps[:mp],
                                w8ss[:, s * CQK + m0: s * CQK + m0 + mp],
                                s8[:, 4, dy:dy + 4, dx:dx + W],
                                start=False, stop=(s == 8))
                        # stage 2 tiles into one 1024-wide store
                        half = (t % 2) * NT
                        if t % 2 == 0:
                            stq[mi] = stagep.tile([128, 2 * NT], BF, tag="stage", name=f"stq{mi}")
                        st = stq[mi]
                        if mi % 2 == 0:
                            nc.vector.tensor_copy(st[:mp, half:half + NT], ps[:mp])
                        else:
                            nc.scalar.copy(st[:mp, half:half + NT], ps[:mp])
                        sq = sqp.tile([128, NT], F32, tag="sq")
                        nc.vector.tensor_mul(sq[:mp], st[:mp, half:half + NT],
                                             st[:mp, half:half + NT])
                        nc.vector.reduce_sum(parts[mi][:mp, t:t + 1], sq[:mp], axis=X)
                        if t % 2 == 1:
                            nc.sync.dma_start(
                                qk[m0:m0 + mp, (t - 1) * NT:(t + 1) * NT], st[:mp])
                    else:           # v0: bf16
                        n_mm = 0
                        for s in range(9):
                            dy, dx = s // 3, s % 3
                            r = (4 * t + dy) % RBP
                            for i in range(5):
                                nc.tensor.matmul(
                                    ps[:mp],
                                    w2vs[i][:, s * CV + m0 - CQK:
                                            s * CV + m0 - CQK + mp],
                                    y1i[i][:, r:r + 4, dx:dx + W],
                                    start=(n_mm == 0), stop=(n_mm == 44))
                                n_mm += 1
                        nc.scalar.copy(v0[:, t * NT:(t + 1) * NT], ps[:])

                # v1 (64 ch): column-tiled concurrent pairs over (t-1, t)
                if t % 2 == 0:
                    ps_v1[0] = psV.tile([128, NT], F32, tag="psv1", name="psv1")
                else:
                    psv = ps_v1[0]
                    n_mm = 0
                    for s in range(9):
                        dy, dx = s // 3, s % 3
                        ra = (4 * (t - 1) + dy) % RBP
                        rb = (4 * t + dy) % RBP
                        for i in range(5):
                            wsl = w2vs[i][:, s * CV + 128: s * CV + 192]
                            nc.tensor.matmul(
                                psv[0:64], wsl, y1i[i][:, ra:ra + 4, dx:dx + W],
                                start=(n_mm == 0), stop=(n_mm == 44),
                                skip_group_check=True)
                            nc.tensor.matmul(
                                psv[64:128], wsl, y1i[i][:, rb:rb + 4, dx:dx + W],
                                start=(n_mm == 0), stop=(n_mm == 44),
                                skip_group_check=True)
                            n_mm += 1
                    cols = (t // 2) * NT
                    nc.vector.tensor_copy(v1t[0:64, cols:cols + NT], psv[0:64])
                    nc.scalar.copy(v1t[64:128, cols:cols + NT], psv[64:128])

                if t >= 1:
                    issue_logits(t - 1)
                # tiles fully read-issued: t's v1 window is still pending
                # until the odd-t pair fires
                consumed = t - 1 if t % 2 == 0 else t
                while (issued < NCH and issued < _chunk_need(t + 2)
                       and not _chunk_conflicts(issued, consumed)):
                    issue_chunk(issued)
                    issued += 1
                assert issued >= _chunk_need(t + 1), (t, issued)
            assert issued == NCH
            issue_logits(NTB - 1)

        # -------- Phase C (small): norms, softmax, FW --------
        with (tc.tile_pool(name="small", bufs=1) as smallp,
              tc.tile_pool(name="soft", bufs=4) as softp,
              tc.tile_pool(name="psF", bufs=1, space="PSUM") as psF):
            wpt_sb = smallp.tile([CH, HEADS * C], BF, tag="wpt")
            nc.sync.dma_start(wpt_sb[:], wpt_d[:, :])
            scs = smallp.tile([CH, HEADS], F32, tag="scs")
            nc.gpsimd.dma_start(out=scs[:],
                                in_=scale_d[0:1, :].to_broadcast((CH, HEADS)))

            for mi, (m0, mp) in enumerate(KT[:3]):
                ssq = smallp.tile([128, 1], F32, tag=f"ssq{mi}", name=f"ssq{mi}")
                nc.vector.reduce_sum(ssq[:mp], parts[mi][:mp, :], axis=X)
                nc.scalar.sqrt(ssq[:mp], ssq[:mp])
                nc.vector.reciprocal(ssq[:mp], ssq[:mp])
                nc.sync.dma_start(rinv_d[0, 128 * mi:128 * mi + mp], ssq[:mp, 0])
                # keep-warm: tiny matmul dependent on ssq
                kw = psF.tile([1, NTB], F32, tag="kw", name=f"kw{mi}")
                nc.tensor.matmul(kw[:], ssq[:mp, 0:1], parts[mi][:mp, :],
                                 start=True, stop=True)

            alpha = smallp.tile([CH, HEADS], F32, tag="alpha")
            for h in range(HEADS):
                nc.sync.dma_start(alpha[:, h:h + 1], rinv_d[0, CH * h:CH * (h + 1)])
            nc.vector.tensor_mul(alpha[:], alpha[:], scs[:])
            ball = smallp.tile([CH, C], F32, tag="ball")
            nc.gpsimd.dma_start(out=ball[:],
                                in_=rinv_d[0:1, C:2 * C].to_broadcast((CH, C)))

            fwt0 = smallp.tile([128, C], BF, tag="fwt0")
            fwt1 = smallp.tile([128, C], BF, tag="fwt1")
            for h in range(HEADS):
                gh = Gall[:, CH * h:CH * (h + 1)]
                nc.vector.tensor_scalar_mul(gh, gh, alpha[:, h:h + 1])
                gsb = softp.tile([CH, CH], F32, tag="gsb")
                nc.vector.tensor_mul(gsb[:], gh, ball[:, CH * h:CH * (h + 1)])
                ex = softp.tile([CH, CH], F32, tag="ex")
                nc.scalar.activation(ex[:], gsb[:],
                                     mybir.ActivationFunctionType.Exp,
                                     scale=1.0)
                sm = softp.tile([CH, 1], F32, tag="sm")
                nc.vector.reduce_sum(sm[:], ex[:], axis=X)
                nc.vector.reciprocal(sm[:], sm[:])
                asb = softp.tile([CH, CH], BF, tag="asb")
                nc.vector.tensor_scalar_mul(asb[:], ex[:], sm[:, 0:1])
                fw_ps = psF.tile([CH, C], F32, tag="fw")
                nc.tensor.matmul(fw_ps[:], asb[:], wpt_sb[:, C * h:C * (h + 1)],
                                 start=True, stop=True)
                fw_sb = softp.tile([CH, C], BF, tag="fwsb")
                nc.any.tensor_copy(fw_sb[:], fw_ps[:])
                # scatter into fwt0 (rows 0:128) / fwt1 (rows 128:192, duplicated
                # at partitions 0:64 and 64:128 for the column-tiled v1 layout)
                r0 = CH * h
                n0 = min(CH, max(0, 128 - r0))
                if n0 > 0:
                    nc.sync.dma_start(fwt0[r0:r0 + n0, :], fw_sb[0:n0, :])
                if n0 < CH:
                    r1 = r0 + n0 - 128
                    nc.sync.dma_start(fwt1[r1:r1 + CH - n0, :], fw_sb[n0:CH, :])
                    nc.sync.dma_start(fwt1[64 + r1:64 + r1 + CH - n0, :],
                                      fw_sb[n0:CH, :])

            # -------- Phase D: out = FW @ v --------
            with (tc.tile_pool(name="ostage", bufs=6) as ostagep,
                  tc.tile_pool(name="psD", bufs=4, space="PSUM") as psD):
                for t in range(NTB):
                    vb = 64 * (t % 2)
                    vcols = (t // 2) * NT
                    for oi, (m0, mp) in enumerate(MT_OUT):
                        ps = psD.tile([128, NT], F32, tag="psD")
                        nc.tensor.matmul(ps[:mp], fwt0[:, m0:m0 + mp],
                                         v0[:, t * NT:(t + 1) * NT],
                                         start=True, stop=False)
                        nc.tensor.matmul(ps[:mp], fwt1[vb:vb + 64, m0:m0 + mp],
                                         v1t[vb:vb + 64, vcols:vcols + NT],
                                         start=False, stop=True)
                        ost = ostagep.tile([128, NT], BF, tag="ost")
                        if (t + oi) % 2 == 0:
                            nc.vector.tensor_copy(ost[:mp], ps[:mp])
                        else:
                            nc.scalar.copy(ost[:mp], ps[:mp])
                        nc.sync.dma_start(out_d[m0:m0 + mp, t * NT:(t + 1) * NT],
                                          ost[:mp])
        psG_ctx.__exit__(None, None, None)


def _prep_shared(w_qkv1, w_qkv2, w_proj, scale):
    w1t = np.zeros((CP2, C3), dtype=BF16NP)
    w1t[:C] = np.ascontiguousarray(w_qkv1[:, :, 0, 0].T).astype(BF16NP)
    w2t = np.transpose(w_qkv2, (2, 3, 1, 0)).reshape(9, C3, C3)          # [s,i,o]

    # bf16 weights for v output channels (384:576): [5, 128, 9*192]
    w2v = np.zeros((5, 128, 9 * CV), dtype=BF16NP)
    for kt, (k0, kp) in enumerate(KT):
        w2v[kt, :kp, :] = np.ascontiguousarray(
            np.transpose(w2t[:, k0:k0 + kp, CQK:], (1, 0, 2)).reshape(kp, 9 * CV)
        ).astype(BF16NP)

    # fp8 weights for q/k output channels (0:384), scaled x64.
    w2qk = w2t[:, :, :CQK] * W8SCALE                                      # [9,576,384]
    w8p = np.zeros((2, 128, 2, 9 * CQK), dtype=F8NP)
    for pp in range(2):
        for j in range(2):
            k0 = 128 * (2 * pp + j)
            w8p[pp, :, j, :] = np.ascontiguousarray(
                np.transpose(w2qk[:, k0:k0 + 128, :], (1, 0, 2)).reshape(128, 9 * CQK)
            ).astype(F8NP)
    w8s = np.zeros((128, 9 * CQK), dtype=F8NP)
    w8s[:64] = np.ascontiguousarray(
        np.transpose(w2qk[:, 512:576, :], (1, 0, 2)).reshape(64, 9 * CQK)
    ).astype(F8NP)

    wpf = w_proj[:, :, 0, 0].T                                            # [c,o]
    wpt = np.concatenate([wpf[h * CH:(h + 1) * CH, :] for h in range(HEADS)],
                         axis=1).astype(BF16NP)                           # [48,768]
    sc = np.asarray(scale, np.float32).reshape(1, HEADS)
    return w1t, w2v, w8p, w8s, wpt, sc


def _make_in_maps(x, w_qkv1, w_qkv2, w_proj, scale):
    w1t, w2v, w8p, w8s, wpt, sc = _prep_shared(
        np.asarray(w_qkv1, np.float32), np.asarray(w_qkv2, np.float32),
        np.asarray(w_proj, np.float32), np.asarray(scale, np.float32))
    x = np.asarray(x, np.float32)
    xp = np.zeros((B, CP2, HP, WP), np.float32)
    xp[:, :C, 1:H + 1, 1:W + 1] = x
    xp = xp.astype(BF16NP).reshape(B, CP2, NPP)
    return [{"xp": xp[i], "w1t": w1t, "w2v": w2v,
             "w8p": w8p.reshape(2, 128, 2 * 9 * CQK), "w8s": w8s,
             "wpt": wpt, "scale": sc}
            for i in range(B)]


def kernel(x, w_qkv1, w_qkv2, w_proj, scale):
    if "nc" not in _CACHE:
        _CACHE["nc"] = _build()
    nc = _CACHE["nc"]
    in_maps = _make_in_maps(x, w_qkv1, w_qkv2, w_proj, scale)
    res = run_bass_kernel_spmd(nc, in_maps, core_ids=list(range(B)))
    out = np.stack([res.results[i]["out"].reshape(C, H, W) for i in range(B)], 0)
    return np.ascontiguousarray(out.astype(np.float32))


# revision 15
# speedup vs baseline: 1.8349x; 1.0363x over previous
"""Trainium2 Bass kernel for channel ("transposed") attention:
  qkv = conv3x3(conv1x1(x)); per-head L2-normalized channel attention; 1x1 proj.

Sharding: pure data-parallel — batch 8 across 8 NeuronCores (one image each).
Per-core pipeline (matmuls accumulate in f32 PSUM):
  A+B fused: conv1x1 chunks stream into a circular SBUF row buffer (19 rows
     x 130 px per input subtile, 16-row period + 3 halo rows) — no DRAM
     round-trip for y1. conv3x3 consumes 6-row windows:
       q/k output channels (0:384): fp8e4 DoubleRow matmuls (2 subtile
       pairs + 1 plain fp8 matmul per shift; weights host-scaled x64
       to dodge e4m3 subnormals — the scale cancels in the L2 norms),
       v channels (384:576): bf16; v0 SBUF-resident; the 64-wide v1 tile
       runs as column-tiled concurrent matmul pairs across two n-tiles.
     q/k streamed to DRAM (1024-wide stores) + squared-norm reduce;
     per-n-tile qk DMA-transposes and per-head logit matmuls interleaved
     (one tile lagged) so the PE stream never breaks.
  C: norms, softmax, FW = wproj @ blockdiag(attn) — all-SBUF plumbing,
     tiny keep-warm matmuls so the PE clock doesn't re-throttle.
  D: out = FW @ v -> bf16 output (host upcasts to f32).
"""
import numpy as np
import ml_dtypes

import concourse.bass as bass
import concourse.tile as tile
from concourse import bacc, mybir
from concourse.bass_utils import run_bass_kernel_spmd

BF16NP = ml_dtypes.bfloat16
F8NP = ml_dtypes.float8_e4m3
BF = mybir.dt.bfloat16
F8 = mybir.dt.float8e4
F32 = mybir.dt.float32
DR = mybir.MatmulPerfMode.DoubleRow

B, C, H, W = 8, 192, 128, 128
HEADS, CH = 4, 48
C3 = 3 * C                      # 576
CP2 = 256                       # 192 padded to 2*128
HP, WP = H + 2, W + 2           # 130
NPIX = H * W                    # 16384
NPP = HP * WP                   # 16900
NT = 512
NTB = NPIX // NT                # 32 conv n-tiles
KT = [(0, 128), (128, 128), (256, 128), (384, 128), (512, 64)]   # 576 split
MT_OUT = [(0, 128), (128, 64)]                                   # out-ch split
CQK = 384                       # q/k output channels
CV = 192                        # v output channels
W8SCALE = 64.0                  # fp8 q/k weight prescale (cancels in L2 norm)

RBP = 16                        # circular buffer period (rows)
RBR = RBP + 3                   # +3 halo rows (dup of rows 0..2)
RB = RBP * WP                   # 2080 px period
HALO = 3 * WP                   # 390 px halo

# phase-A chunks: 33 at 512*c covering [0,16896), plus one at NPP-512
A_OFFS = [512 * c for c in range(33)] + [NPP - NT]
NCH = len(A_OFFS)               # 34


def _chunk_need(t):
    """chunks that must be issued before conv tile t (pixels < 520*t+780)."""
    if t >= NTB:
        return NCH
    need = 0
    cover = 520 * t + 780
    for c, off in enumerate(A_OFFS):
        if off < cover:
            need = c + 1
    return need


def _chunk_conflicts(c, consumed):
    """True if chunk c's circular-buffer write would clobber a window still
    needed by a tile whose reads haven't been issued yet (> consumed).
    Chunk c evicts the pixels one buffer-lap back: [off-RB, off-RB+NT)."""
    lo, hi = A_OFFS[c] - RB, A_OFFS[c] - RB + NT
    for tp in range(max(consumed + 1, 0), NTB):
        w0 = 520 * tp
        if w0 < hi and lo < w0 + 780:
            return True
    return False


_CACHE = {}


def _build():
    nc = bacc.Bacc("TRN2", target_bir_lowering=False, debug=False, num_devices=8)
    xp_d = nc.dram_tensor("xp", [CP2, NPP], BF, kind="ExternalInput").ap()
    w1t_d = nc.dram_tensor("w1t", [CP2, C3], BF, kind="ExternalInput").ap()
    w2v_d = nc.dram_tensor("w2v", [5, 128, 9 * CV], BF, kind="ExternalInput").ap()
    w8p_d = nc.dram_tensor("w8p", [2, 128, 2 * 9 * CQK], F8, kind="ExternalInput").ap()
    w8s_d = nc.dram_tensor("w8s", [128, 9 * CQK], F8, kind="ExternalInput").ap()
    wpt_d = nc.dram_tensor("wpt", [CH, HEADS * C], BF, kind="ExternalInput").ap()
    scale_d = nc.dram_tensor("scale", [1, HEADS], F32, kind="ExternalInput").ap()
    out_d = nc.dram_tensor("out", [C, NPIX], BF, kind="ExternalOutput").ap()

    with tile.TileContext(nc) as tc:
        with tc.tile_pool(name="dram", bufs=1, space="DRAM") as dram:
            qk = dram.tile([2 * C, NPIX], BF)
            rinv_d = dram.tile([1, 512], F32)
            _build_body(nc, tc, xp_d, w1t_d, w2v_d, w8p_d, w8s_d, wpt_d,
                        scale_d, out_d, qk, rinv_d)
    nc.compile()
    return nc


def _build_body(nc, tc, xp_d, w1t_d, w2v_d, w8p_d, w8s_d, wpt_d, scale_d,
                out_d, qk, rinv_d):
    X = mybir.AxisListType.X

    with tc.tile_pool(name="persist", bufs=1) as persist:
        v0 = persist.tile([128, NPIX], BF, tag="v0")
        # v1 (64 ch): even tiles on partitions 0:64, odd on 64:128
        v1t = persist.tile([128, NPIX // 2], BF, tag="v1t")
        parts = [persist.tile([mp, NTB], F32, tag=f"part{i}", name=f"part{i}")
                 for i, (m0, mp) in enumerate(KT[:3])]
        # circular y1 row buffers, one per input subtile
        y1b = [persist.tile([128, RBR * WP], BF,
                            tag=f"y1b{i}", name=f"y1b{i}") for i in range(5)]
        y1i = [b.rearrange("p (r w) -> p r w", r=RBR) for b in y1b]
        psG_ctx = tc.tile_pool(name="psG", bufs=1, space="PSUM")
        psG = psG_ctx.__enter__()

        with (tc.tile_pool(name="wts", bufs=1) as wts,
              tc.tile_pool(name="xk", bufs=8) as xkp,
              tc.tile_pool(name="slab8", bufs=3) as slab8p,
              tc.tile_pool(name="stage", bufs=6) as stagep,
              tc.tile_pool(name="sq", bufs=3) as sqp,
              tc.tile_pool(name="qkt", bufs=8) as qktp,
              tc.tile_pool(name="psA", bufs=5, space="PSUM") as psA,
              tc.tile_pool(name="psV", bufs=2, space="PSUM") as psV):

            w1s = []
            for i in range(2):
                t = wts.tile([128, C3], BF, tag=f"w1_{i}", name=f"w1_{i}")
                nc.sync.dma_start(t[:], w1t_d[128 * i:128 * (i + 1), :])
                w1s.append(t)
            w2vs = [wts.tile([128, 9 * CV], BF,
                             tag=f"w2v_{i}", name=f"w2v_{i}") for i in range(5)]
            w8ps = [wts.tile([128, 2, 9 * CQK], F8, tag=f"w8p_{i}", name=f"w8p_{i}")
                    for i in range(2)]
            w8ss = wts.tile([128, 9 * CQK], F8, tag="w8s", name="w8s")

            def load_phaseB_weights():
                for i in range(5):
                    nc.sync.dma_start(w2vs[i][:], w2v_d[i, :, :])
                for i in range(2):
                    nc.sync.dma_start(w8ps[i][:], w8p_d[i, :, :])
                nc.sync.dma_start(w8ss[:], w8s_d[:, :])

            # ---- phase-A chunk issuer: y1 = w1 @ xp into circular buffer ----
            xk_big = {}

            def issue_chunk(c):
                off = A_OFFS[c]
                if c < 32:
                    big = c // 2
                    if big not in xk_big:
                        boff = 1024 * big
                        xs = []
                        for i in range(2):
                            xk = xkp.tile([128, 1024], BF, tag="xk")
                            nc.sync.dma_start(
                                xk[:], xp_d[128 * i:128 * (i + 1), boff:boff + 1024])
                            xs.append(xk)
                        xk_big[big] = xs
                        if len(xk_big) > 3:
                            del xk_big[min(xk_big)]
                    xs = xk_big[big]
                    half = (c % 2) * 512
                    mov = [x[:, half:half + 512] for x in xs]
                else:
                    xs = []
                    for i in range(2):
                        xk = xkp.tile([128, NT], BF, tag="xks", name="xks")
                        nc.sync.dma_start(xk[:], xp_d[128 * i:128 * (i + 1),
                                                      off:off + NT])
                        xs.append(xk)
                    mov = [x[:] for x in xs]

                # circular positions + halo duplication pieces
                pos = off % RB
                pieces = []
                for s0, s1 in ([(0, RB - pos), (RB - pos, NT)]
                               if pos + NT > RB else [(0, NT)]):
                    p0 = (pos + s0) % RB
                    pieces.append((p0, s0, s1))
                    if p0 < HALO:        # duplicate into halo rows
                        e = min(HALO - p0, s1 - s0)
                        pieces.append((p0 + RB, s0, s0 + e))

                for mi, (m0, mp) in enumerate(KT):
                    ps = psA.tile([128, NT], F32, tag="ps")
                    for i in range(2):
                        nc.tensor.matmul(ps[:mp], w1s[i][:, m0:m0 + mp],
                                         mov[i], start=(i == 0), stop=(i == 1))
                    for pi, (p0, s0, s1) in enumerate(pieces):
                        if (mi + pi) % 2 == 0:
                            nc.vector.tensor_copy(
                                y1b[mi][:mp, p0:p0 + (s1 - s0)], ps[:mp, s0:s1])
                        else:
                            nc.scalar.copy(
                                y1b[mi][:mp, p0:p0 + (s1 - s0)], ps[:mp, s0:s1])

            # logits PSUM accumulator: 4 heads packed in one bank [48, 192]
            Gall = psG.tile([CH, HEADS * CH], F32, tag="Gall")

            def issue_logits(tt):
                for j in range(4 * tt, 4 * tt + 4):
                    qkt = qktp.tile([128, 2 * C], BF, tag="qkt", name="qkt")
                    nc.sync.dma_start_transpose(qkt[:], qk[:, j * 128:(j + 1) * 128])
                    for h in range(HEADS):
                        nc.tensor.matmul(
                            Gall[:, CH * h:CH * (h + 1)],
                            qkt[:, CH * h:CH * (h + 1)],
                            qkt[:, C + CH * h:C + CH * (h + 1)],
                            start=(j == 0 and h == 0),
                            stop=(j == 127 and h == HEADS - 1),
                            skip_group_check=True)

            # zero rows 64:128 of subtile-4 buffer once (K padding so every
            # matmul is a uniform K=128 — partial-K matmuls defeat the
            # LDWEIGHTS prefetch and cost ~100ns each)
            nc.vector.memset(y1b[4][64:128, :], 0.0)
            # preload ACT sqrt+exp tables now so phase C skips the 1.3us loads
            tpre = wts.tile([1, 4], F32, tag="tpre")
            nc.vector.memset(tpre[:], 1.0)
            nc.scalar.sqrt(tpre[:], tpre[:])
            nc.scalar.activation(tpre[:], tpre[:],
                                 mybir.ActivationFunctionType.Exp, scale=1.0)

            # ---- fused pipeline ----
            for c in range(_chunk_need(1)):
                issue_chunk(c)
            load_phaseB_weights()
            issued = _chunk_need(1)
            stq = [None] * 3
            ps_v1 = [None]

            for t in range(NTB):
                r6 = (4 * t) % RBP
                # fp8 conversion of the 6-row window, all 5 subtiles
                s8 = slab8p.tile([128, 5, 6, WP], F8, tag="s8")
                for i in range(5):
                    if i % 2 == 0:
                        nc.vector.tensor_copy(s8[:, i], y1i[i][:, r6:r6 + 6, :])
                    else:
                        nc.scalar.copy(s8[:, i], y1i[i][:, r6:r6 + 6, :])

                for mi, (m0, mp) in enumerate(KT[:4]):
                    ps = psA.tile([128, NT], F32, tag="ps")
                    if mi < 3:      # q/k: fp8 DoubleRow
                        n_mm = 0
                        for s in range(9):
                            dy, dx = s // 3, s % 3
                            for pp in range(2):
                                nc.tensor.matmul(
                                    ps[:mp],
                                    w8ps[pp][:, :, s * CQK + m0: s * CQK + m0 + mp],
                                    s8[:, 2 * pp:2 * pp + 2, dy:dy + 4, dx:dx + W],
                                    start=(n_mm == 0), stop=False,
                                    perf_mode=DR)
                                n_mm += 1
                            nc.tensor.matmul(
                                ps[:mp],
                                w8ss[:, s * CQK + m0: s * CQK + m0 + mp],
                                s8[:, 4, dy:dy + 4, dx:dx + W],
                                start=False, stop=(s == 8))
                        # stage 2 tiles into one 1024-wide store
                        half = (t % 2) * NT
                        if t % 2 == 0:
                            stq[mi] = stagep.tile([128, 2 * NT], BF, tag="stage", name=f"stq{mi}")
                        st = stq[mi]
                        if mi % 2 == 0:
                            nc.vector.tensor_copy(st[:mp, half:half + NT], ps[:mp])
                        else:
                            nc.scalar.copy(st[:mp, half:half + NT], ps[:mp])
                        sq = sqp.tile([128, NT], F32, tag="sq")
                        nc.vector.tensor_mul(sq[:mp], st[:mp, half:half + NT],
                                             st[:mp, half:half + NT])
                        nc.vector.reduce_sum(parts[mi][:mp, t:t + 1], sq[:mp], axis=X)
                        if t % 2 == 1:
                            nc.sync.dma_start(
                                qk[m0:m0 + mp, (t - 1) * NT:(t + 1) * NT], st[:mp])
                    else:           # v0: bf16
                        n_mm = 0
                        for s in range(9):
                            dy, dx = s // 3, s % 3
                            r = (4 * t + dy) % RBP
                            for i in range(5):
                                nc.tensor.matmul(
                                    ps[:mp],
                                    w2vs[i][:, s * CV + m0 - CQK:
                                            s * CV + m0 - CQK + mp],
                                    y1i[i][:, r:r + 4, dx:dx + W],
                                    start=(n_mm == 0), stop=(n_mm == 44))
                                n_mm += 1
                        nc.scalar.copy(v0[:, t * NT:(t + 1) * NT], ps[:])

                # v1 (64 ch): column-tiled concurrent pairs over (t-1, t)
                if t % 2 == 0:
                    ps_v1[0] = psV.tile([128, NT], F32, tag="psv1", name="psv1")
                else:
                    psv = ps_v1[0]
                    n_mm = 0
                    for s in range(9):
                        dy, dx = s // 3, s % 3
                        ra = (4 * (t - 1) + dy) % RBP
                        rb = (4 * t + dy) % RBP
                        for i in range(5):
                            wsl = w2vs[i][:, s * CV + 128: s * CV + 192]
                            nc.tensor.matmul(
                                psv[0:64], wsl, y1i[i][:, ra:ra + 4, dx:dx + W],
                                start=(n_mm == 0), stop=(n_mm == 44),
                                skip_group_check=True)
                            nc.tensor.matmul(
                                psv[64:128], wsl, y1i[i][:, rb:rb + 4, dx:dx + W],
                                start=(n_mm == 0), stop=(n_mm == 44),
                                skip_group_check=True)
                            n_mm += 1
                    cols = (t // 2) * NT
                    nc.vector.tensor_copy(v1t[0:64, cols:cols + NT], psv[0:64])
                    nc.scalar.copy(v1t[64:128, cols:cols + NT], psv[64:128])

                if t >= 1:
                    issue_logits(t - 1)
                # tiles fully read-issued: t's v1 window is still pending
                # until the odd-t pair fires
                consumed = t - 1 if t % 2 == 0 else t
                while (issued < NCH and issued < _chunk_need(t + 2)
                       and not _chunk_conflicts(issued, consumed)):
                    issue_chunk(issued)
                    issued += 1
                assert issued >= _chunk_need(t + 1), (t, issued)
            assert issued == NCH
            issue_logits(NTB - 1)

        # -------- Phase C (small): norms, softmax, FW --------
        with (tc.tile_pool(name="small", bufs=1) as smallp,
              tc.tile_pool(name="soft", bufs=4) as softp,
              tc.tile_pool(name="psF", bufs=1, space="PSUM") as psF):
            wpt_sb = smallp.tile([CH, HEADS * C], BF, tag="wpt")
            nc.sync.dma_start(wpt_sb[:], wpt_d[:, :])
            ones48 = smallp.tile([1, CH], F32, tag="ones48")
            nc.vector.memset(ones48[:], 1.0)
            scrow = smallp.tile([1, HEADS], F32, tag="scrow")
            nc.scalar.dma_start(scrow[:], scale_d[0:1, :])
            rrow = smallp.tile([1, 2 * C], F32, tag="rrow")

            for mi, (m0, mp) in enumerate(KT[:3]):
                ssq = smallp.tile([128, 1], F32, tag=f"ssq{mi}", name=f"ssq{mi}")
                nc.vector.reduce_sum(ssq[:mp], parts[mi][:mp, :], axis=X)
                nc.scalar.sqrt(ssq[:mp], ssq[:mp])
                nc.vector.reciprocal(ssq[:mp], ssq[:mp])
                nc.scalar.dma_start(rinv_d[0, 128 * mi:128 * mi + mp], ssq[:mp, 0])
                # keep-warm: tiny matmul dependent on ssq
                kw = psF.tile([1, NTB], F32, tag="kw", name=f"kw{mi}")
                nc.tensor.matmul(kw[:], ssq[:mp, 0:1], parts[mi][:mp, :],
                                 start=True, stop=True)
            nc.scalar.dma_start(rrow[:], rinv_d[0:1, 0:2 * C])

            # partition-broadcasts as K=1 ones-matmuls (no gpsimd: its software
            # DGE path costs a ~9us DRAIN on the critical path)
            bps = psF.tile([CH, C + HEADS], F32, tag="bps")
            nc.tensor.matmul(bps[:, 0:C], ones48[:], rrow[0:1, C:2 * C],
                             start=True, stop=True)
            nc.tensor.matmul(bps[:, C:], ones48[:], scrow[0:1, :],
                             start=True, stop=True, skip_group_check=True)
            ball = smallp.tile([CH, C], F32, tag="ball")
            nc.vector.tensor_copy(ball[:], bps[:, 0:C])

            alpha = smallp.tile([CH, HEADS], F32, tag="alpha")
            for h in range(HEADS):
                nc.scalar.dma_start(alpha[:, h:h + 1],
                                    rinv_d[0, CH * h:CH * (h + 1)])
            nc.vector.tensor_mul(alpha[:], alpha[:], bps[:, C:])

            fwt0 = smallp.tile([128, C], BF, tag="fwt0")
            # fwt1 variants padded to K=128: even tiles use rows 0:64 (rest 0),
            # odd tiles rows 64:128 — so phase D never runs a K=64 matmul
            fwt1e = smallp.tile([128, C], BF, tag="fwt1e")
            fwt1o = smallp.tile([128, C], BF, tag="fwt1o")
            nc.vector.memset(fwt1e[64:128, :], 0.0)
            nc.vector.memset(fwt1o[0:64, :], 0.0)
            for h in range(HEADS):
                gh = Gall[:, CH * h:CH * (h + 1)]
                nc.vector.tensor_scalar_mul(gh, gh, alpha[:, h:h + 1])
                gsb = softp.tile([CH, CH], F32, tag="gsb")
                nc.vector.tensor_mul(gsb[:], gh, ball[:, CH * h:CH * (h + 1)])
                ex = softp.tile([CH, CH], F32, tag="ex")
                nc.scalar.activation(ex[:], gsb[:],
                                     mybir.ActivationFunctionType.Exp,
                                     scale=1.0)
                sm = softp.tile([CH, 1], F32, tag="sm")
                nc.vector.reduce_sum(sm[:], ex[:], axis=X)
                nc.vector.reciprocal(sm[:], sm[:])
                asb = softp.tile([CH, CH], BF, tag="asb")
                nc.vector.tensor_scalar_mul(asb[:], ex[:], sm[:, 0:1])
                fw_ps = psF.tile([CH, C], F32, tag="fw")
                nc.tensor.matmul(fw_ps[:], asb[:], wpt_sb[:, C * h:C * (h + 1)],
                                 start=True, stop=True)
                fw_sb = softp.tile([CH, C], BF, tag="fwsb")
                nc.any.tensor_copy(fw_sb[:], fw_ps[:])
                # scatter into fwt0 (rows 0:128) / fwt1 (rows 128:192, duplicated
                # at partitions 0:64 and 64:128 for the column-tiled v1 layout)
                r0 = CH * h
                n0 = min(CH, max(0, 128 - r0))
                if n0 > 0:
                    nc.scalar.dma_start(fwt0[r0:r0 + n0, :], fw_sb[0:n0, :])
                if n0 < CH:
                    r1 = r0 + n0 - 128
                    nc.scalar.dma_start(fwt1e[r1:r1 + CH - n0, :], fw_sb[n0:CH, :])
                    nc.scalar.dma_start(fwt1o[64 + r1:64 + r1 + CH - n0, :],
                                        fw_sb[n0:CH, :])

            # -------- Phase D: out = FW @ v --------
            with (tc.tile_pool(name="ostage", bufs=6) as ostagep,
                  tc.tile_pool(name="psD", bufs=4, space="PSUM") as psD):
                osts = [None, None]
                for t in range(NTB):
                    fwt1x = fwt1e if t % 2 == 0 else fwt1o
                    vcols = (t // 2) * NT
                    for oi, (m0, mp) in enumerate(MT_OUT):
                        ps = psD.tile([128, NT], F32, tag="psD")
                        nc.tensor.matmul(ps[:mp], fwt0[:, m0:m0 + mp],
                                         v0[:, t * NT:(t + 1) * NT],
                                         start=True, stop=False)
                        nc.tensor.matmul(ps[:mp], fwt1x[:, m0:m0 + mp],
                                         v1t[:, vcols:vcols + NT],
                                         start=False, stop=True)
                        if t % 2 == 0:
                            osts[oi] = ostagep.tile([128, 2 * NT], BF, tag="ost",
                                                    name=f"ost{oi}")
                        ost = osts[oi]
                        half = (t % 2) * NT
                        if (t + oi) % 2 == 0:
                            nc.vector.tensor_copy(ost[:mp, half:half + NT], ps[:mp])
                        else:
                            nc.scalar.copy(ost[:mp, half:half + NT], ps[:mp])
                        if t % 2 == 1:
                            nc.sync.dma_start(
                                out_d[m0:m0 + mp, (t - 1) * NT:(t + 1) * NT],
                                ost[:mp])
        psG_ctx.__exit__(None, None, None)


def _prep_shared(w_qkv1, w_qkv2, w_proj, scale):
    w1t = np.zeros((CP2, C3), dtype=BF16NP)
    w1t[:C] = np.ascontiguousarray(w_qkv1[:, :, 0, 0].T).astype(BF16NP)
    w2t = np.transpose(w_qkv2, (2, 3, 1, 0)).reshape(9, C3, C3)          # [s,i,o]

    # bf16 weights for v output channels (384:576): [5, 128, 9*192]
    w2v = np.zeros((5, 128, 9 * CV), dtype=BF16NP)
    for kt, (k0, kp) in enumerate(KT):
        w2v[kt, :kp, :] = np.ascontiguousarray(
            np.transpose(w2t[:, k0:k0 + kp, CQK:], (1, 0, 2)).reshape(kp, 9 * CV)
        ).astype(BF16NP)

    # fp8 weights for q/k output channels (0:384), scaled x64.
    w2qk = w2t[:, :, :CQK] * W8SCALE                                      # [9,576,384]
    w8p = np.zeros((2, 128, 2, 9 * CQK), dtype=F8NP)
    for pp in range(2):
        for j in range(2):
            k0 = 128 * (2 * pp + j)
            w8p[pp, :, j, :] = np.ascontiguousarray(
                np.transpose(w2qk[:, k0:k0 + 128, :], (1, 0, 2)).reshape(128, 9 * CQK)
            ).astype(F8NP)
    w8s = np.zeros((128, 9 * CQK), dtype=F8NP)
    w8s[:64] = np.ascontiguousarray(
        np.transpose(w2qk[:, 512:576, :], (1, 0, 2)).reshape(64, 9 * CQK)
    ).astype(F8NP)

    wpf = w_proj[:, :, 0, 0].T                                            # [c,o]
    wpt = np.concatenate([wpf[h * CH:(h + 1) * CH, :] for h in range(HEADS)],
                         axis=1).astype(BF16NP)                           # [48,768]
    sc = np.asarray(scale, np.float32).reshape(1, HEADS)
    return w1t, w2v, w8p, w8s, wpt, sc


def _make_in_maps(x, w_qkv1, w_qkv2, w_proj, scale):
    w1t, w2v, w8p, w8s, wpt, sc = _prep_shared(
        np.asarray(w_qkv1, np.float32), np.asarray(w_qkv2, np.float32),
        np.asarray(w_proj, np.float32), np.asarray(scale, np.float32))
    x = np.asarray(x, np.float32)
    xp = np.zeros((B, CP2, HP, WP), np.float32)
    xp[:, :C, 1:H + 1, 1:W + 1] = x
    xp = xp.astype(BF16NP).reshape(B, CP2, NPP)
    return [{"xp": xp[i], "w1t": w1t, "w2v": w2v,
             "w8p": w8p.reshape(2, 128, 2 * 9 * CQK), "w8s": w8s,
             "wpt": wpt, "scale": sc}
            for i in range(B)]


def kernel(x, w_qkv1, w_qkv2, w_proj, scale):
    if "nc" not in _CACHE:
        _CACHE["nc"] = _build()
    nc = _CACHE["nc"]
    in_maps = _make_in_maps(x, w_qkv1, w_qkv2, w_proj, scale)
    res = run_bass_kernel_spmd(nc, in_maps, core_ids=list(range(B)))
    out = np.stack([res.results[i]["out"].reshape(C, H, W) for i in range(B)], 0)
    return np.ascontiguousarray(out.astype(np.float32))
